# revision 24
# baseline (speedup 1.0000x reference)
"""T5-style encoder self-attention (dense_transformer) on 8 Trainium2 NeuronCores.

Problem (full shapes): hidden [2,2048,2048], Wq/Wk/Wv/Wo [2048,2048],
rel_emb [32,32] (bidirectional T5 relative-position bias), mask [2,1,1,2048].

Sharding: data-parallel over batch (2) x tensor-parallel over heads (4 groups
of 8 heads) = 8 cores, Megatron-style. Each core computes a partial output
[2048,2048] for its batch (its 8 heads through its Wo row-slice); the host
sums 4 partials per batch.

Per-core kernel design (bf16 operands, fp32 PSUM accumulation):
  - The relative-position bias is applied MULTIPLICATIVELY for every tile:
    px = exp(s/8 + mask) * erel, where erel = exp(bias) is a host-computed
    [8 heads, 4096 diagonals] bf16 table read through a Toeplitz shear view
    (partition stride 1, free stride 1).  Host numerics sim puts the
    all-multiplicative absmax-rel at 7.7e-3 (vs 6.0e-3 for the old scheme
    that identity-injected near-diagonal tiles into PSUM; the inject matmuls
    cost ~38us of PE issue time and a 16us identity DMA on the critical
    startup path).
  - Phase B: single pass over x^T computes pair-0 Q^T/K^T and V for ALL
    heads (6 matmuls per x^T chunk, PE-bound).  Q^T is stored with s
    REVERSED so the bias becomes a positive-shear Toeplitz.
  - Phase C attention, per (head-pair, q-chunk), k-tile loop pipelined one
    iteration ahead:
      * the two per-head QK matmuls are packed as concurrent 64-row-group
        tiles (tile_position (0,0)/(64,0));
      * ACT computes exp(s/8 + mask) in one [128,1024] shot per k-tile;
        DVE multiplies by the erel shear slice (far AND near tiles);
      * next-pair Q/K projection matmuls are interleaved PER k-tile so they
        fill the PE's ACT-wait bubbles; their x^T tiles are group-loaded
        (4 k-chunks per DMA, 4KB partition lines) and prefetched one group
        ahead so the proj matmuls never wait on DMA.
  - x^T is host-tiled to [128, NQC, NDT, 512] so every [128, 4, 512] group
    load has 4KB contiguous per-partition lines (the old [D,S] layout gave
    1KB lines, which capped each DMA queue near 85 GB/s and stalled the
    interleaved projections).
  - V augmentation: per pair, even head block = [v(0:64) | ones(64)] (M=65,
    denominator lands on PSUM partition 64), odd head block = 128 wide with
    ones at col 32 and v at cols 64:128 (denominator on partition 32, ctx on
    partitions 64:128), keeping every normalize op partition-aligned.
  - Normalize is DEFERRED and PE-free: cx evacuates to SBUF at qc end
    (freeing its PSUM slot), then one qc later a DVE+DMA-only chain runs:
    pack denominator rows to a base-0 tile (custom DVE ops require base
    partition 0), reciprocal_approx_fast, bounce the two reciprocal rows
    through DRAM, stride-0 DMAs broadcast them across partitions, and fused
    DVE tensor_tensors do normalize + un-reverse + bf16 writeback.
  - The next qc's first score-group is pre-emitted in the current qc's tail
    (exactly one PSUM slot is free there) so ACT never idles at boundaries.
  - Startup: the first x^T group and the first wq/wk/wv chunk are the FIRST
    DMAs on their queues (sync / gpsimd), so the first matmul fires ~9us in
    instead of ~41us; mask + ACT-table warmup + wo ride the scalar queue.
  - Phase D output projection: descending s-tiles (low tiles depend on the
    last deferred normalize), m looped inside nd so consecutive matmuls hit
    different PSUM banks; evacuation alternates ACT/DVE; the two out DMAs
    per s-tile alternate sync/gpsimd queues.
"""

import math
import sys

for _p in ("/opt/trn_rl_repo",):
    if _p not in sys.path:
        sys.path.insert(0, _p)

import numpy as np

import concourse.bass as bass
import concourse.mybir as mybir
import concourse.tile as tile
from concourse import bacc
from concourse.bass_utils import run_bass_kernel_spmd

DT = mybir.dt
AF = mybir.ActivationFunctionType
OP = mybir.AluOpType

# ---- problem constants (hardcoded per contract) ----
B, S, D = 2, 2048, 2048
N_HEADS, D_KV = 32, 64
NUM_BUCKETS, MAX_DISTANCE = 32, 128
NCORES = 8
HL = 8            # heads per core
P = 128
SC = 512          # free-dim chunk
NKT = S // P      # 16 k-tiles
NQC = S // SC     # 4 q-chunks
NDT = D // P      # 16 D-tiles
NMT = (HL * D_KV) // P   # 4 hd m-tiles per core
NPAIR = HL // 2   # 4 head pairs per core
NDIAG = 4096
W_U = 3968        # erel shear tile width (covers all diagonals any tile hits)
VW = 193          # vaug per-(kt,pair) width: even block 65 + odd block 128
NKG = NDT // 4    # 4 kd-groups of 4 chunks per q-chunk (x^T group loads)


def _rel_bucket_host(d):
    """Exact numpy replica of reference._relative_position_bucket."""
    num_buckets = NUM_BUCKETS // 2          # 16
    max_exact = num_buckets // 2            # 8
    rel = np.asarray(d, dtype=np.int64)
    buckets = (rel > 0).astype(np.int32) * num_buckets
    arel = np.abs(rel)
    is_small = arel < max_exact
    rp_safe = np.maximum(arel, 1).astype(np.float32)
    log_ratio = np.log(rp_safe / np.float32(max_exact)).astype(np.float32)
    scale = np.float32(math.log(MAX_DISTANCE / max_exact))
    rp_large = max_exact + (log_ratio / scale * np.float32(num_buckets - max_exact)).astype(np.int32)
    rp_large = np.minimum(rp_large, num_buckets - 1)
    buckets = buckets + np.where(is_small, arel.astype(np.int32), rp_large)
    return buckets.astype(np.int32)


def _bias_table(rel_emb_slice):
    """rel_emb_slice: [NUM_BUCKETS, HL] fp32 -> erel [HL, NDIAG] bf16,
    erel[h, i] = exp(bias(d = i - 2047)); erel[:, 4095] is never read."""
    import ml_dtypes
    i = np.arange(NDIAG - 1)
    b = _rel_bucket_host(i - (S - 1))                  # [4095]
    vals = rel_emb_slice[b, :]                         # [4095, HL] fp32
    erel = np.zeros((HL, NDIAG), dtype=np.float32)
    erel[:, : NDIAG - 1] = np.exp(vals.T)
    return erel.astype(ml_dtypes.bfloat16)


def _build():
    nc = bacc.Bacc(None, name="attn_tp")

    # x^T host-tiled: xt[p, qc, kd, j] = x[qc*512+j, kd*128+p], so a
    # [128, 4, 512] kd-group load is one DMA with 4KB per-partition lines
    xt = nc.declare_dram_parameter("xt", [P, NQC, NDT, SC], DT.bfloat16,
                                   isOutput=False)
    # weights arrive HOST-SHUFFLED to [p][kt][h] so per-partition lines are
    # contiguous multi-KB runs (DMA packet rate is the limiter at 1KB lines)
    wq = nc.declare_dram_parameter("wq", [P, NDT * HL * D_KV], DT.bfloat16, isOutput=False)
    wk = nc.declare_dram_parameter("wk", [P, NDT * HL * D_KV], DT.bfloat16, isOutput=False)
    wv = nc.declare_dram_parameter("wv", [P, NDT * HL * D_KV], DT.bfloat16, isOutput=False)
    wo = nc.declare_dram_parameter("wo", [P, NMT * D], DT.bfloat16, isOutput=False)
    mask = nc.declare_dram_parameter("mask", [S], DT.float32, isOutput=False)
    erel = nc.declare_dram_parameter("erel", [HL, NDIAG], DT.bfloat16, isOutput=False)
    out = nc.declare_dram_parameter("out", [S, D], DT.float32, isOutput=True)

    with tile.TileContext(nc) as tc:
        with (
            tc.tile_pool(name="res", bufs=1) as res,          # persistent tensors
            tc.tile_pool(name="xtp", bufs=3) as xtp,          # x^T groups (sync q)
            tc.tile_pool(name="upool", bufs=2) as upool,      # exp-bias shear tiles
            tc.tile_pool(name="pexp", bufs=3) as pexpp,       # probs tiles
            tc.tile_pool(name="stage", bufs=2) as stage,      # normalize staging
            tc.tile_pool(name="outp", bufs=3) as outp,        # out staging
            tc.tile_pool(name="psum", bufs=4, space="PSUM") as psum,  # [128,1024] slots
            tc.tile_pool(name="dram", bufs=2, space="DRAM") as dramp,
        ):
            # ---------- constants / resident tensors ----------
            mask_sb = res.tile([P, NKT], DT.float32, tag="mask")
            # mask + ACT exp-table warm-up ride the (otherwise idle) scalar
            # queue so the sync/gpsimd queues start with the critical loads
            nc.scalar.dma_start(mask_sb[:], mask.ap().rearrange("(kt p) -> p kt", p=P))

            wq_sb = res.tile([P, NDT, HL * D_KV], DT.bfloat16, tag="wq")
            wk_sb = res.tile([P, NDT, HL * D_KV], DT.bfloat16, tag="wk")
            wv_sb = res.tile([P, NDT, HL * D_KV], DT.bfloat16, tag="wv")
            wo_sb = res.tile([P, NMT, D], DT.bfloat16, tag="wo")

            # persistent activations.  qt/kt/ctxt are split per (pair,
            # q-chunk) so the tile dep tracker never serializes score /
            # phase-D reads behind a LATER chunk's drain writes (the
            # tracker is not interval-precise across a big tensor).
            qt_pq = [[res.tile([P, SC], DT.bfloat16, tag=f"qt{m}_{c}",
                               name=f"qt{m}_{c}") for c in range(NQC)]
                     for m in range(NMT)]                      # q REVERSED
            kt_pq = [[res.tile([P, SC], DT.bfloat16, tag=f"kt{m}_{c}",
                               name=f"kt{m}_{c}") for c in range(NQC)]
                     for m in range(NMT)]
            vaug = res.tile([P, NKT, NPAIR, VW], DT.bfloat16, tag="vaug")
            ctxt_pq = [[res.tile([P, SC], DT.bfloat16, tag=f"ct{m}_{c}",
                                 name=f"ct{m}_{c}") for c in range(NQC)]
                       for m in range(NMT)]
            # only the two ones-columns are ever read outside the V blocks
            # (psum rows other than the denominator rows are never consumed)
            nc.vector.memset(vaug[:, :, :, 64:65], 1.0)
            nc.vector.memset(vaug[:, :, :, 97:98], 1.0)

            # ACT exp table warm-up (hide the ~2.7us table load under phase B)
            warm = res.tile([1, 2], DT.float32, tag="warm")
            nc.scalar.activation(out=warm[0:1, 0:1], in_=mask_sb[0:1, 0:1], func=AF.Exp)

            def rev_chunk(t):
                """reversed-q view over one [rows, SC] chunk tile: writing
                reversed data lands in natural order.  A logical q-chunk qc
                written reversed covers NATURAL chunk NQC-1-qc."""
                return bass.AP(
                    tensor=t.tensor,
                    offset=t.offset + (SC - 1),
                    ap=[list(t.ap[0]), [-1, SC]],
                )

            def load_u(pr, eng=None):
                """erel shear tile [P, 2, W_U] for pair pr: u[p, i, w] =
                erel[2*pr+i, p + w].  One DMA per head: fewer, larger DMAs
                keep the issuing queue free (the scheduler bunches split
                loads into queue-hogging bursts anyway)."""
                u = upool.tile([P, 2, W_U], DT.bfloat16, tag="u",
                               name=f"u{pr}", bufs=2)
                ap0 = erel.ap()
                for i, hh in enumerate((2 * pr, 2 * pr + 1)):
                    shear = bass.AP(
                        tensor=ap0.tensor,
                        offset=ap0.offset + hh * NDIAG,
                        ap=[[1, P], [1, W_U]],
                    )
                    (eng or nc.gpsimd).dma_start(u[:, i, :], shear)
                return u

            def load_wchunk(g, eng=None):
                """one 4-kd chunk of weights; chunk 0 rides sync (HWDGE,
                ~0.6us first byte) right behind the first x group so the
                first matmuls fire ~10us in; later chunks stream on gpsimd
                in kd order."""
                eng = eng or nc.gpsimd
                cw = HL * D_KV
                c0, c1 = g * 4 * cw, (g + 1) * 4 * cw
                eng.dma_start(wq_sb[:, g * 4:(g + 1) * 4, :], wq[:, c0:c1])
                eng.dma_start(wk_sb[:, g * 4:(g + 1) * 4, :], wk[:, c0:c1])
                eng.dma_start(wv_sb[:, g * 4:(g + 1) * 4, :], wv[:, c0:c1])

            def load_xgroup(nq, g):
                """one [128, 4, 512] x^T kd-group (4KB partition lines)."""
                t = xtp.tile([P, 4, SC], DT.bfloat16, tag="xt",
                             name=f"xg{nq}_{g}")
                nc.sync.dma_start(t[:], xt[:, nq, 4 * g:4 * (g + 1), :])
                return t

            # ---------- phase B: pair-0 Q/K + V (all heads), single x^T pass ----
            wc0_loaded = False
            for nq in range(NQC):
                qk_ps = psum.tile([P, 2 * SC], DT.float32, tag="ps",
                                  name=f"qkps0_{nq}")
                q_ps, k_ps = qk_ps[:, 0:SC], qk_ps[:, SC:2 * SC]
                v01 = psum.tile([P, 2 * SC], DT.float32, tag="ps", name=f"v01_{nq}")
                v23 = psum.tile([P, 2 * SC], DT.float32, tag="ps", name=f"v23_{nq}")
                v_ps = [v01[:, 0:SC], v01[:, SC:2 * SC],
                        v23[:, 0:SC], v23[:, SC:2 * SC]]
                for g in range(NKG):
                    if nq == 0 and g == 0:
                        # per-kd loads: the first matmul only needs 128KB of
                        # x and 3x128KB of weights, so don't gate it on the
                        # full 512KB group + 1.5MB chunk
                        xg = xtp.tile([P, 4, SC], DT.bfloat16, tag="xt",
                                      name="xg0_0")
                        cw = HL * D_KV
                        for c in range(4):
                            nc.sync.dma_start(xg[:, c, :], xt[:, 0, c, :])
                            if not wc0_loaded:
                                for wsb, wsrc in ((wq_sb, wq), (wk_sb, wk),
                                                  (wv_sb, wv)):
                                    nc.sync.dma_start(
                                        wsb[:, c:c + 1, :],
                                        wsrc[:, c * cw:(c + 1) * cw])
                        wc0_loaded = True
                    else:
                        xg = load_xgroup(nq, g)
                    if nq == 0 and g + 1 < NKG:
                        load_wchunk(g + 1)   # prefetch next weight chunk
                    for c in range(4):
                        kd = 4 * g + c
                        xt_t = xg[:, c, :]
                        nc.tensor.matmul(
                            q_ps, wq_sb[:, kd, 0:P], xt_t,
                            start=(kd == 0), stop=(kd == NDT - 1),
                        )
                        nc.tensor.matmul(
                            k_ps, wk_sb[:, kd, 0:P], xt_t,
                            start=(kd == 0), stop=(kd == NDT - 1),
                        )
                        for st in range(4):
                            nc.tensor.matmul(
                                v_ps[st], xg[:, c, st * P:(st + 1) * P],
                                wv_sb[:, kd, :],
                                start=(kd == 0), stop=(kd == NDT - 1),
                            )
                if nq == 0:
                    # pair-0 u table behind the weight chunks on gpsimd
                    # (phase C only needs it ~90us in)
                    u0 = load_u(0)
                # drain: V -> vaug blocks first (frees the 2 V psum slots the
                # next nq's V matmuls are waiting on), then q/k casts
                for st in range(4):
                    ktg = nq * 4 + st
                    vsrc = v_ps[st].rearrange("p (pr par d) -> p pr par d",
                                              par=2, d=D_KV)
                    nc.vector.tensor_copy(vaug[:, ktg, :, 0:D_KV],
                                          vsrc[:, :, 0, :])
                    nc.vector.tensor_copy(vaug[:, ktg, :, 129:193],
                                          vsrc[:, :, 1, :])
                nc.vector.tensor_copy(rev_chunk(qt_pq[0][NQC - 1 - nq][:, :]),
                                      q_ps)
                nc.vector.tensor_copy(kt_pq[0][nq][:, :], k_ps)

            # wo load issues from the idle scalar queue once phase B's
            # critical streams are done (needed only in phase D)
            nc.scalar.dma_start(wo_sb.rearrange("p a b -> p (a b)"), wo[:])

            # ---------- phase C: attention, proj of pair pr+1 interleaved ----
            def emit_sg(pr, qc, kt):
                """scores psum group for (pair, q-chunk, k-tile): the two
                heads run as concurrent 64-row-group tiles."""
                jg0 = qc * SC
                s01 = psum.tile([P, 2 * SC], DT.float32, tag="ps",
                                name=f"s{pr}_{qc}_{kt}")
                kc, ko = kt // 4, (kt % 4) * P
                nc.tensor.matmul(
                    s01[:, 0:SC], kt_pq[pr][kc][0:64, ko:ko + P],
                    qt_pq[pr][qc][0:64, :],
                    start=True, stop=True, tile_position=(0, 0),
                )
                nc.tensor.matmul(
                    s01[:, SC:2 * SC], kt_pq[pr][kc][64:128, ko:ko + P],
                    qt_pq[pr][qc][64:128, :],
                    start=True, stop=True, tile_position=(64, 0),
                )
                return s01

            # proj x^T group tiles, prefetched one group ahead (keyed by
            # group index within the current (proj, qc))
            def load_pgroup(proj, qc, g):
                t = xtp.tile([P, 4, SC], DT.bfloat16, tag="xt",
                             name=f"xp{proj}_{qc}_{g}")
                nc.sync.dma_start(t[:], xt[:, qc, 4 * g:4 * (g + 1), :])
                return t

            def attn_qc(pr, qc, u_t, proj, pending, s_pre, nxt_sg, pg0,
                        upf=None, px_pre=None):
                """attention for head pair pr, reversed-q chunk qc.
                proj: None or pr+1 (emit that pair's Q/K proj, 1 kd per kt).
                pg0: pre-loaded x^T group 0 for the proj (or None).
                Returns (normalize closure, pre-emitted next score group,
                pre-loaded group 0 for the NEXT (proj, qc), next-pair u)."""
                u_ret = None
                h0, h1 = 2 * pr, 2 * pr + 1
                jg0 = qc * SC
                cx01 = psum.tile([P, 2 * SC], DT.float32, tag="ps",
                                 name=f"cx{pr}_{qc}")
                if proj is not None:
                    pj_ps = psum.tile([P, 2 * SC], DT.float32, tag="ps",
                                      name=f"pjps{proj}_{qc}")
                    pjq, pjk = pj_ps[:, 0:SC], pj_ps[:, SC:2 * SC]
                    pgs = {0: pg0}

                def emit_proj(kt):
                    g, c = kt // 4, kt % 4
                    if c == 0 and g + 1 < NKG:
                        pgs[g + 1] = load_pgroup(proj, qc, g + 1)
                    kd = kt
                    xt_t = pgs[g][:, c, :]
                    nc.tensor.matmul(
                        pjq, wq_sb[:, kd, proj * P:(proj + 1) * P], xt_t,
                        start=(kd == 0), stop=(kd == NDT - 1),
                    )
                    nc.tensor.matmul(
                        pjk, wk_sb[:, kd, proj * P:(proj + 1) * P], xt_t,
                        start=(kd == 0), stop=(kd == NDT - 1),
                    )

                # 2-deep software pipeline: s(kt+2) is emitted before PV(kt)
                # so the in-order PE queue keeps a backlog (hides LDWEIGHTS
                # and cross-engine semaphore latency).  pending() emits the
                # PREVIOUS qc's deferred normalize chain (DVE+DMA only).
                sq = [s_pre if s_pre is not None else emit_sg(pr, qc, 0),
                      emit_sg(pr, qc, 1)]
                for kt in range(NKT):
                    if kt + 2 < NKT:
                        sq.append(emit_sg(pr, qc, kt + 2))
                    if proj is not None:
                        emit_proj(kt)
                    if kt == 2 and pending is not None:
                        pending[0]()
                    if kt == 8 and upf is not None:
                        u_ret = upf()
                    if kt == 12 and pending is not None:
                        pending[1]()
                    s01 = sq[kt]
                    if kt == 0 and px_pre is not None:
                        px = px_pre
                    else:
                        px = pexpp.tile([P, 2 * SC], DT.bfloat16, tag="pexp",
                                        name=f"px{pr}_{qc}_{kt}")
                        nc.scalar.activation(
                            out=px[:], in_=s01[:], func=AF.Exp,
                            bias=mask_sb[:, kt:kt + 1],
                            scale=1.0 / math.sqrt(D_KV),
                        )
                        j0 = kt * P + jg0
                        nc.vector.tensor_tensor(
                            px.rearrange("p (h j) -> p h j", h=2),
                            px.rearrange("p (h j) -> p h j", h=2),
                            u_t[:, :, j0:j0 + SC], OP.mult
                        )
                    nc.tensor.matmul(
                        cx01[0:65, 0:SC], vaug[:, kt, pr, 0:65], px[:, 0:SC],
                        start=(kt == 0), stop=(kt == NKT - 1),
                    )
                    nc.tensor.matmul(
                        cx01[:, SC:2 * SC], vaug[:, kt, pr, 65:VW],
                        px[:, SC:2 * SC],
                        start=(kt == 0), stop=(kt == NKT - 1),
                    )

                # proj drain (reversed q for qt)
                if proj is not None:
                    nc.scalar.copy(rev_chunk(qt_pq[proj][NQC - 1 - qc][:, :]),
                                   pjq)
                    nc.vector.tensor_copy(kt_pq[proj][qc][:, :], pjk)

                # prefetch group 0 of the NEXT (proj, qc)'s x^T
                pg_next = None
                if qc + 1 < NQC and proj is not None:
                    pg_next = load_pgroup(proj, qc + 1, 0)
                elif qc == NQC - 1 and proj is not None and proj + 1 < NPAIR:
                    pg_next = load_pgroup(proj + 1, 0, 0)

                # pre-emit the NEXT qc's first score group AND its exp +
                # erel multiply, so the boundary Vector burst (proj drain +
                # cx evacuation) never delays the next qc's first PV
                s_next = None
                px_next = None
                if nxt_sg is not None:
                    s_next, npr, nqc, nu = nxt_sg()
                    px_next = pexpp.tile([P, 2 * SC], DT.bfloat16,
                                         tag="pexp", name=f"pxp{npr}_{nqc}")
                    nc.scalar.activation(
                        out=px_next[:], in_=s_next[:], func=AF.Exp,
                        bias=mask_sb[:, 0:1], scale=1.0 / math.sqrt(D_KV),
                    )
                    nc.vector.tensor_tensor(
                        px_next.rearrange("p (h j) -> p h j", h=2),
                        px_next.rearrange("p (h j) -> p h j", h=2),
                        nu[:, :, nqc * SC:nqc * SC + SC], OP.mult
                    )

                # ---- evacuate cx to SBUF (frees the PSUM slot), then the
                # rest of normalize+writeback is DEFERRED into the next qc
                # (DVE + DMA only; the PE never touches it) ----
                cxs = stage.tile([P, 2 * SC], DT.bfloat16, tag="cxs",
                                 name=f"cxs{pr}_{qc}", bufs=1)
                nc.vector.tensor_copy(cxs[:], cx01[:])

                bc_box = {}

                def norm_a():
                    # denominators: h0 on row 64 (cols 0:512), h1 on row 32
                    # (cols 512:1024).  Custom DVE ops need base-partition-0
                    # operands, so pack both rows into a base-0 tile first.
                    # Runs at kt==2 of the NEXT qc; the DRAM-bounce broadcast
                    # DMAs get ~8 k-tiles of latency slack before norm_b's
                    # tensor_tensors (at kt==8) consume bc_sb - the ~5us
                    # chain latency never blocks the Vector queue.
                    dnf = stage.tile([P, SC], DT.float32, tag="dnf",
                                     name=f"dnf{pr}_{qc}", bufs=1)
                    nc.vector.tensor_copy(dnf[64:65, :], cxs[64:65, 0:SC])
                    nc.vector.tensor_copy(dnf[32:33, :], cxs[32:33, SC:2 * SC])
                    rb = stage.tile([P, SC], DT.float32, tag="rb",
                                    name=f"rb{pr}_{qc}", bufs=1)
                    nc.vector.reciprocal_approx_fast(out=rb[:], in_=dnf[:])
                    rbh = stage.tile([P, SC], DT.bfloat16, tag="rbh",
                                     name=f"rbh{pr}_{qc}", bufs=1)
                    nc.vector.tensor_copy(rbh[64:65, :], rb[64:65, :])
                    nc.vector.tensor_copy(rbh[32:33, :], rb[32:33, :])
                    # broadcast across partitions: bounce the reciprocal
                    # rows through DRAM, then stride-0 reads replicate them.
                    # norm_b consumes bc_sb only at kt==12, so the DMA chain
                    # has ~13us of slack before it can block Vector.
                    bnc = dramp.tile([2, SC], DT.bfloat16, tag="bnc",
                                     name=f"bnc{pr}_{qc}")
                    nc.sync.dma_start(bnc[0:1, :], rbh[64:65, :])
                    nc.sync.dma_start(bnc[1:2, :], rbh[32:33, :])
                    bc_sb = stage.tile([P, SC], DT.bfloat16, tag="bc",
                                       name=f"bcs{pr}_{qc}", bufs=2)
                    src0 = bass.AP(tensor=bnc.tensor, offset=bnc.offset,
                                   ap=[[0, 64], [1, SC]])
                    src1 = bass.AP(tensor=bnc.tensor, offset=bnc.offset + SC,
                                   ap=[[0, 64], [1, SC]])
                    nc.gpsimd.dma_start(bc_sb[0:64, :], src0)
                    nc.gpsimd.dma_start(bc_sb[64:128, :], src1)
                    # (writes ride sync/HWDGE: the SWDGE write completion
                    # was taking ~8us under load, starving the read chain)
                    bc_box["bc"] = bc_sb

                def norm_b():
                    bc_sb = bc_box["bc"]
                    ct = ctxt_pq[pr][NQC - 1 - qc]
                    nc.vector.tensor_tensor(
                        rev_chunk(ct[0:64, :]),
                        cxs[0:64, 0:SC], bc_sb[0:64, :], OP.mult)
                    nc.vector.tensor_tensor(
                        rev_chunk(ct[64:128, :]),
                        cxs[64:128, SC:2 * SC], bc_sb[64:128, :], OP.mult)
                return (norm_a, norm_b), s_next, px_next, pg_next, u_ret

            u_t = u0  # pair-0 table already loaded during phase B
            pending = None
            s_pre = None
            px_pre = None
            pg0 = load_pgroup(1, 0, 0)
            seq = [(pr, qc) for pr in range(NPAIR) for qc in range(NQC)]
            next_u = None
            for idx, (pr, qc) in enumerate(seq):
                nxt = pr + 1 if pr + 1 < NPAIR else None
                if nxt is not None and qc == 0:
                    upf = (lambda nxt=nxt: load_u(nxt))
                else:
                    upf = None
                if idx + 1 < len(seq):
                    npr, nqc = seq[idx + 1]
                    # the u table the NEXT (pr,qc) will multiply with: at a
                    # pair boundary it is the freshly loaded next_u
                    nxt_sg = (lambda npr=npr, nqc=nqc:
                              (emit_sg(npr, nqc, 0), npr, nqc,
                               u_t if npr == pr else next_u))
                else:
                    nxt_sg = None
                pending, s_pre, px_pre, pg0, u_ret = attn_qc(
                    pr, qc, u_t, nxt, pending, s_pre, nxt_sg, pg0, upf,
                    px_pre)
                if u_ret is not None:
                    next_u = u_ret
                if qc == NQC - 1 and nxt is not None:
                    u_t = next_u
                    next_u = None
            pending[0]()
            pending[1]()

            # ---------- phase D: output projection (descending st: the
            # low-st tiles depend on the last deferred normalize) ----------
            for st in range(NKT - 1, -1, -1):
                oa = psum.tile([P, 2 * SC], DT.float32, tag="ps",
                               name=f"oa{st}")
                ob = psum.tile([P, 2 * SC], DT.float32, tag="ps",
                               name=f"ob{st}")
                o_ps = [oa[:, 0:SC], oa[:, SC:2 * SC],
                        ob[:, 0:SC], ob[:, SC:2 * SC]]
                for m in range(NMT):
                    for nd in range(NQC):
                        nc.tensor.matmul(
                            o_ps[nd],
                            ctxt_pq[m][st // 4][:, (st % 4) * P:
                                                (st % 4) * P + P],
                            wo_sb[:, m, nd * SC:(nd + 1) * SC],
                            start=(m == 0), stop=(m == NMT - 1),
                        )
                for half in range(2):
                    # separate staging tiles per engine: a shared tile
                    # serialized the vector copy behind the scalar copy
                    o_a = outp.tile([P, SC], DT.float32, tag="outa",
                                    name=f"ota{st}_{half}")
                    o_b = outp.tile([P, SC], DT.float32, tag="outb",
                                    name=f"otb{st}_{half}")
                    nc.scalar.copy(o_a[:], o_ps[2 * half])
                    nc.vector.tensor_copy(o_b[:], o_ps[2 * half + 1])
                    eng = nc.sync if half == 0 else nc.gpsimd
                    c0 = half * 2 * SC
                    eng.dma_start(
                        out[st * P:(st + 1) * P, c0:c0 + SC], o_a[:])
                    eng.dma_start(
                        out[st * P:(st + 1) * P, c0 + SC:c0 + 2 * SC], o_b[:])

    nc.finalize()
    return nc


_NC_CACHE = None


def _get_nc():
    global _NC_CACHE
    if _NC_CACHE is None:
        _NC_CACHE = _build()
    return _NC_CACHE


def _in_maps(hidden_states, attention_mask, Wq, Wk, Wv, Wo, rel_emb):
    import ml_dtypes
    bf16 = ml_dtypes.bfloat16
    maps = []
    for c in range(NCORES):
        b, g = c // 4, c % 4
        hlo, hhi = g * HL, (g + 1) * HL
        erel = _bias_table(
            np.ascontiguousarray(rel_emb[:, hlo:hhi], dtype=np.float32))
        def shuf(w):  # [NDT*P, C] -> [P, NDT*C] partition-contiguous
            cc = w.shape[1]
            return np.ascontiguousarray(
                w.reshape(-1, P, cc).transpose(1, 0, 2).reshape(P, -1))
        # xt[p, qc, kd, j] = x[qc*512+j, kd*128+p]
        xtt = np.ascontiguousarray(
            hidden_states[b].reshape(NQC, SC, NDT, P).transpose(3, 0, 2, 1)
        ).astype(bf16)
        maps.append({
            "xt": xtt,
            "wq": shuf(Wq[:, hlo * D_KV:hhi * D_KV]).astype(bf16),
            "wk": shuf(Wk[:, hlo * D_KV:hhi * D_KV]).astype(bf16),
            "wv": shuf(Wv[:, hlo * D_KV:hhi * D_KV]).astype(bf16),
            "wo": shuf(Wo[hlo * D_KV:hhi * D_KV, :]).astype(bf16),
            "mask": np.ascontiguousarray(attention_mask[b, 0, 0, :]).astype(np.float32),
            "erel": erel,
        })
    return maps


def kernel(hidden_states, attention_mask, Wq, Wk, Wv, Wo, rel_emb, _trace=False,
           _trace_kwargs=None):
    hidden_states = np.asarray(hidden_states, dtype=np.float32)
    attention_mask = np.asarray(attention_mask, dtype=np.float32)
    Wq = np.asarray(Wq, dtype=np.float32)
    Wk = np.asarray(Wk, dtype=np.float32)
    Wv = np.asarray(Wv, dtype=np.float32)
    Wo = np.asarray(Wo, dtype=np.float32)
    rel_emb = np.asarray(rel_emb, dtype=np.float32)

    nc = _get_nc()
    maps = _in_maps(hidden_states, attention_mask, Wq, Wk, Wv, Wo, rel_emb)
    kw = dict(_trace_kwargs or {})
    res = run_bass_kernel_spmd(nc, maps, core_ids=list(range(NCORES)),
                               trace=_trace, **kw)
    kernel.last_results = res
    outp = np.empty((B, S, D), dtype=np.float32)
    for b in range(B):
        acc = np.asarray(res.results[4 * b]["out"], dtype=np.float32).copy()
        for g in range(1, 4):
            acc += np.asarray(res.results[4 * b + g]["out"], dtype=np.float32)
        outp[b] = acc
    return outp


# revision 25
# speedup vs baseline: 1.0278x; 1.0278x over previous
"""T5-style encoder self-attention (dense_transformer) on 8 Trainium2 NeuronCores.

Problem (full shapes): hidden [2,2048,2048], Wq/Wk/Wv/Wo [2048,2048],
rel_emb [32,32] (bidirectional T5 relative-position bias), mask [2,1,1,2048].

Sharding: data-parallel over batch (2) x tensor-parallel over heads (4 groups
of 8 heads) = 8 cores, Megatron-style. Each core computes a partial output
[2048,2048] for its batch (its 8 heads through its Wo row-slice); the host
sums 4 partials per batch.

Per-core kernel design (bf16 operands, fp32 PSUM accumulation):
  - The relative-position bias is applied MULTIPLICATIVELY for every tile:
    px = exp(s/8 + mask) * erel, where erel = exp(bias) is a host-computed
    [8 heads, 4096 diagonals] bf16 table read through a Toeplitz shear view
    (partition stride 1, free stride 1).  Host numerics sim puts the
    all-multiplicative absmax-rel at 7.7e-3 (vs 6.0e-3 for the old scheme
    that identity-injected near-diagonal tiles into PSUM; the inject matmuls
    cost ~38us of PE issue time and a 16us identity DMA on the critical
    startup path).
  - Phase B: single pass over x^T computes pair-0 Q^T/K^T and V for ALL
    heads (6 matmuls per x^T chunk, PE-bound).  Q^T is stored with s
    REVERSED so the bias becomes a positive-shear Toeplitz.
  - Phase C attention, per (head-pair, q-chunk), k-tile loop pipelined one
    iteration ahead:
      * the two per-head QK matmuls are packed as concurrent 64-row-group
        tiles (tile_position (0,0)/(64,0));
      * ACT computes exp(s/8 + mask) in one [128,1024] shot per k-tile;
        DVE multiplies by the erel shear slice (far AND near tiles);
      * next-pair Q/K projection matmuls are interleaved PER k-tile so they
        fill the PE's ACT-wait bubbles; their x^T tiles are group-loaded
        (4 k-chunks per DMA, 4KB partition lines) and prefetched one group
        ahead so the proj matmuls never wait on DMA.
  - x^T is host-tiled to [128, NQC, NDT, 512] so every [128, 4, 512] group
    load has 4KB contiguous per-partition lines (the old [D,S] layout gave
    1KB lines, which capped each DMA queue near 85 GB/s and stalled the
    interleaved projections).
  - V augmentation: per pair, even head block = [v(0:64) | ones(64)] (M=65,
    denominator lands on PSUM partition 64), odd head block = 128 wide with
    ones at col 32 and v at cols 64:128 (denominator on partition 32, ctx on
    partitions 64:128), keeping every normalize op partition-aligned.
  - Normalize is DEFERRED and PE-free: cx evacuates to SBUF at qc end
    (freeing its PSUM slot), then one qc later a DVE+DMA-only chain runs:
    pack denominator rows to a base-0 tile (custom DVE ops require base
    partition 0), reciprocal_approx_fast, bounce the two reciprocal rows
    through DRAM, stride-0 DMAs broadcast them across partitions, and fused
    DVE tensor_tensors do normalize + un-reverse + bf16 writeback.
  - The next qc's first score-group is pre-emitted in the current qc's tail
    (exactly one PSUM slot is free there) so ACT never idles at boundaries.
  - Startup: the first x^T group and the first wq/wk/wv chunk are the FIRST
    DMAs on their queues (sync / gpsimd), so the first matmul fires ~9us in
    instead of ~41us; mask + ACT-table warmup + wo ride the scalar queue.
  - Phase D output projection: descending s-tiles (low tiles depend on the
    last deferred normalize), m looped inside nd so consecutive matmuls hit
    different PSUM banks; evacuation alternates ACT/DVE; the two out DMAs
    per s-tile alternate sync/gpsimd queues.
"""

import math
import sys

for _p in ("/opt/trn_rl_repo",):
    if _p not in sys.path:
        sys.path.insert(0, _p)

import numpy as np

import concourse.bass as bass
import concourse.mybir as mybir
import concourse.tile as tile
from concourse import bacc
from concourse.bass_utils import run_bass_kernel_spmd

DT = mybir.dt
AF = mybir.ActivationFunctionType
OP = mybir.AluOpType

# ---- problem constants (hardcoded per contract) ----
B, S, D = 2, 2048, 2048
N_HEADS, D_KV = 32, 64
NUM_BUCKETS, MAX_DISTANCE = 32, 128
NCORES = 8
HL = 8            # heads per core
P = 128
SC = 512          # free-dim chunk
NKT = S // P      # 16 k-tiles
NQC = S // SC     # 4 q-chunks
NDT = D // P      # 16 D-tiles
NMT = (HL * D_KV) // P   # 4 hd m-tiles per core
NPAIR = HL // 2   # 4 head pairs per core
NDIAG = 4096
W_U = 3968        # erel shear tile width (covers all diagonals any tile hits)
VW = 193          # vaug per-(kt,pair) width: even block 65 + odd block 128
NKG = NDT // 4    # 4 kd-groups of 4 chunks per q-chunk (x^T group loads)


def _rel_bucket_host(d):
    """Exact numpy replica of reference._relative_position_bucket."""
    num_buckets = NUM_BUCKETS // 2          # 16
    max_exact = num_buckets // 2            # 8
    rel = np.asarray(d, dtype=np.int64)
    buckets = (rel > 0).astype(np.int32) * num_buckets
    arel = np.abs(rel)
    is_small = arel < max_exact
    rp_safe = np.maximum(arel, 1).astype(np.float32)
    log_ratio = np.log(rp_safe / np.float32(max_exact)).astype(np.float32)
    scale = np.float32(math.log(MAX_DISTANCE / max_exact))
    rp_large = max_exact + (log_ratio / scale * np.float32(num_buckets - max_exact)).astype(np.int32)
    rp_large = np.minimum(rp_large, num_buckets - 1)
    buckets = buckets + np.where(is_small, arel.astype(np.int32), rp_large)
    return buckets.astype(np.int32)


def _bias_table(rel_emb_slice):
    """rel_emb_slice: [NUM_BUCKETS, HL] fp32 -> erel [HL, NDIAG] bf16,
    erel[h, i] = exp(bias(d = i - 2047)); erel[:, 4095] is never read."""
    import ml_dtypes
    i = np.arange(NDIAG - 1)
    b = _rel_bucket_host(i - (S - 1))                  # [4095]
    vals = rel_emb_slice[b, :]                         # [4095, HL] fp32
    erel = np.zeros((HL, NDIAG), dtype=np.float32)
    erel[:, : NDIAG - 1] = np.exp(vals.T)
    return erel.astype(ml_dtypes.bfloat16)


def _build():
    nc = bacc.Bacc(None, name="attn_tp")

    # x^T host-tiled: xt[p, qc, kd, j] = x[qc*512+j, kd*128+p], so a
    # [128, 4, 512] kd-group load is one DMA with 4KB per-partition lines
    xt = nc.declare_dram_parameter("xt", [P, NQC, NDT, SC], DT.bfloat16,
                                   isOutput=False)
    # weights arrive HOST-SHUFFLED to [p][kt][h] so per-partition lines are
    # contiguous multi-KB runs (DMA packet rate is the limiter at 1KB lines)
    wq = nc.declare_dram_parameter("wq", [P, NDT * HL * D_KV], DT.bfloat16, isOutput=False)
    wk = nc.declare_dram_parameter("wk", [P, NDT * HL * D_KV], DT.bfloat16, isOutput=False)
    wv = nc.declare_dram_parameter("wv", [P, NDT * HL * D_KV], DT.bfloat16, isOutput=False)
    wo = nc.declare_dram_parameter("wo", [P, NMT * D], DT.bfloat16, isOutput=False)
    mask = nc.declare_dram_parameter("mask", [S], DT.float32, isOutput=False)
    erel = nc.declare_dram_parameter("erel", [HL, NDIAG], DT.bfloat16, isOutput=False)
    out = nc.declare_dram_parameter("out", [S, D], DT.float32, isOutput=True)

    with tile.TileContext(nc) as tc:
        with (
            tc.tile_pool(name="res", bufs=1) as res,          # persistent tensors
            tc.tile_pool(name="xtp", bufs=3) as xtp,          # x^T groups (sync q)
            tc.tile_pool(name="upool", bufs=2) as upool,      # exp-bias shear tiles
            tc.tile_pool(name="pexp", bufs=3) as pexpp,       # probs tiles
            tc.tile_pool(name="stage", bufs=2) as stage,      # normalize staging
            tc.tile_pool(name="outp", bufs=3) as outp,        # out staging
            tc.tile_pool(name="psum", bufs=4, space="PSUM") as psum,  # [128,1024] slots
            tc.tile_pool(name="dram", bufs=2, space="DRAM") as dramp,
        ):
            # ---------- constants / resident tensors ----------
            mask_sb = res.tile([P, NKT], DT.float32, tag="mask")
            # mask + ACT exp-table warm-up ride the (otherwise idle) scalar
            # queue so the sync/gpsimd queues start with the critical loads
            nc.scalar.dma_start(mask_sb[:], mask.ap().rearrange("(kt p) -> p kt", p=P))

            wq_sb = res.tile([P, NDT, HL * D_KV], DT.bfloat16, tag="wq")
            wk_sb = res.tile([P, NDT, HL * D_KV], DT.bfloat16, tag="wk")
            wv_sb = res.tile([P, NDT, HL * D_KV], DT.bfloat16, tag="wv")
            wo_sb = res.tile([P, NMT, D], DT.bfloat16, tag="wo")

            # persistent activations.  qt/kt/ctxt are split per (pair,
            # q-chunk) so the tile dep tracker never serializes score /
            # phase-D reads behind a LATER chunk's drain writes (the
            # tracker is not interval-precise across a big tensor).
            qt_pq = [[res.tile([P, SC], DT.bfloat16, tag=f"qt{m}_{c}",
                               name=f"qt{m}_{c}") for c in range(NQC)]
                     for m in range(NMT)]                      # q REVERSED
            kt_pq = [[res.tile([P, SC], DT.bfloat16, tag=f"kt{m}_{c}",
                               name=f"kt{m}_{c}") for c in range(NQC)]
                     for m in range(NMT)]
            vaug = res.tile([P, NKT, NPAIR, VW], DT.bfloat16, tag="vaug")
            ctxt_pq = [[res.tile([P, SC], DT.bfloat16, tag=f"ct{m}_{c}",
                                 name=f"ct{m}_{c}") for c in range(NQC)]
                       for m in range(NMT)]
            # only the two ones-columns are ever read outside the V blocks
            # (psum rows other than the denominator rows are never consumed)
            nc.vector.memset(vaug[:, :, :, 64:65], 1.0)
            nc.vector.memset(vaug[:, :, :, 97:98], 1.0)

            # ACT exp table warm-up (hide the ~2.7us table load under phase B)
            warm = res.tile([1, 2], DT.float32, tag="warm")
            nc.scalar.activation(out=warm[0:1, 0:1], in_=mask_sb[0:1, 0:1], func=AF.Exp)

            def rev_chunk(t):
                """reversed-q view over one [rows, SC] chunk tile: writing
                reversed data lands in natural order.  A logical q-chunk qc
                written reversed covers NATURAL chunk NQC-1-qc."""
                return bass.AP(
                    tensor=t.tensor,
                    offset=t.offset + (SC - 1),
                    ap=[list(t.ap[0]), [-1, SC]],
                )

            def load_u(pr, eng=None):
                """erel shear tile [P, 2, W_U] for pair pr: u[p, i, w] =
                erel[2*pr+i, p + w].  One DMA per head: fewer, larger DMAs
                keep the issuing queue free (the scheduler bunches split
                loads into queue-hogging bursts anyway)."""
                u = upool.tile([P, 2, W_U], DT.bfloat16, tag="u",
                               name=f"u{pr}", bufs=2)
                ap0 = erel.ap()
                for i, hh in enumerate((2 * pr, 2 * pr + 1)):
                    shear = bass.AP(
                        tensor=ap0.tensor,
                        offset=ap0.offset + hh * NDIAG,
                        ap=[[1, P], [1, W_U]],
                    )
                    (eng or nc.gpsimd).dma_start(u[:, i, :], shear)
                return u

            def load_wchunk(g, eng=None):
                """one 4-kd chunk of weights; chunk 0 rides sync (HWDGE,
                ~0.6us first byte) right behind the first x group so the
                first matmuls fire ~10us in; later chunks stream on gpsimd
                in kd order."""
                eng = eng or nc.gpsimd
                cw = HL * D_KV
                c0, c1 = g * 4 * cw, (g + 1) * 4 * cw
                eng.dma_start(wq_sb[:, g * 4:(g + 1) * 4, :], wq[:, c0:c1])
                eng.dma_start(wk_sb[:, g * 4:(g + 1) * 4, :], wk[:, c0:c1])
                eng.dma_start(wv_sb[:, g * 4:(g + 1) * 4, :], wv[:, c0:c1])

            def load_xgroup(nq, g):
                """one [128, 4, 512] x^T kd-group (4KB partition lines)."""
                t = xtp.tile([P, 4, SC], DT.bfloat16, tag="xt",
                             name=f"xg{nq}_{g}")
                nc.sync.dma_start(t[:], xt[:, nq, 4 * g:4 * (g + 1), :])
                return t

            # ---------- phase B: pair-0 Q/K + V (all heads), single x^T pass ----
            wc0_loaded = False
            for nq in range(NQC):
                qk_ps = psum.tile([P, 2 * SC], DT.float32, tag="ps",
                                  name=f"qkps0_{nq}")
                q_ps, k_ps = qk_ps[:, 0:SC], qk_ps[:, SC:2 * SC]
                v01 = psum.tile([P, 2 * SC], DT.float32, tag="ps", name=f"v01_{nq}")
                v23 = psum.tile([P, 2 * SC], DT.float32, tag="ps", name=f"v23_{nq}")
                v_ps = [v01[:, 0:SC], v01[:, SC:2 * SC],
                        v23[:, 0:SC], v23[:, SC:2 * SC]]
                for g in range(NKG):
                    xg = load_xgroup(nq, g)
                    if nq == 0 and not wc0_loaded:
                        load_wchunk(0, eng=nc.sync)
                        wc0_loaded = True
                    if nq == 0 and g + 1 < NKG:
                        load_wchunk(g + 1)   # prefetch next weight chunk
                    for c in range(4):
                        kd = 4 * g + c
                        xt_t = xg[:, c, :]
                        nc.tensor.matmul(
                            q_ps, wq_sb[:, kd, 0:P], xt_t,
                            start=(kd == 0), stop=(kd == NDT - 1),
                        )
                        nc.tensor.matmul(
                            k_ps, wk_sb[:, kd, 0:P], xt_t,
                            start=(kd == 0), stop=(kd == NDT - 1),
                        )
                        for st in range(4):
                            nc.tensor.matmul(
                                v_ps[st], xg[:, c, st * P:(st + 1) * P],
                                wv_sb[:, kd, :],
                                start=(kd == 0), stop=(kd == NDT - 1),
                            )
                if nq == 0:
                    # pair-0 u table behind the weight chunks on gpsimd
                    # (phase C only needs it ~90us in)
                    u0 = load_u(0)
                # drain: V -> vaug blocks first (frees the 2 V psum slots the
                # next nq's V matmuls are waiting on), then q/k casts
                for st in range(4):
                    ktg = nq * 4 + st
                    vsrc = v_ps[st].rearrange("p (pr par d) -> p pr par d",
                                              par=2, d=D_KV)
                    nc.vector.tensor_copy(vaug[:, ktg, :, 0:D_KV],
                                          vsrc[:, :, 0, :])
                    nc.vector.tensor_copy(vaug[:, ktg, :, 129:193],
                                          vsrc[:, :, 1, :])
                nc.vector.tensor_copy(rev_chunk(qt_pq[0][NQC - 1 - nq][:, :]),
                                      q_ps)
                nc.vector.tensor_copy(kt_pq[0][nq][:, :], k_ps)

            # wo load issues from the idle scalar queue once phase B's
            # critical streams are done (needed only in phase D)
            nc.scalar.dma_start(wo_sb.rearrange("p a b -> p (a b)"), wo[:])

            # ---------- phase C: attention, proj of pair pr+1 interleaved ----
            def emit_sg(pr, qc, kt):
                """scores psum group for (pair, q-chunk, k-tile): the two
                heads run as concurrent 64-row-group tiles."""
                jg0 = qc * SC
                s01 = psum.tile([P, 2 * SC], DT.float32, tag="ps",
                                name=f"s{pr}_{qc}_{kt}")
                kc, ko = kt // 4, (kt % 4) * P
                nc.tensor.matmul(
                    s01[:, 0:SC], kt_pq[pr][kc][0:64, ko:ko + P],
                    qt_pq[pr][qc][0:64, :],
                    start=True, stop=True, tile_position=(0, 0),
                )
                nc.tensor.matmul(
                    s01[:, SC:2 * SC], kt_pq[pr][kc][64:128, ko:ko + P],
                    qt_pq[pr][qc][64:128, :],
                    start=True, stop=True, tile_position=(64, 0),
                )
                return s01

            # proj x^T group tiles, prefetched one group ahead (keyed by
            # group index within the current (proj, qc))
            def load_pgroup(proj, qc, g):
                t = xtp.tile([P, 4, SC], DT.bfloat16, tag="xt",
                             name=f"xp{proj}_{qc}_{g}")
                nc.sync.dma_start(t[:], xt[:, qc, 4 * g:4 * (g + 1), :])
                return t

            def attn_qc(pr, qc, u_t, proj, pending, s_pre, nxt_sg, pg0,
                        upf=None, px_pre=None):
                """attention for head pair pr, reversed-q chunk qc.
                proj: None or pr+1 (emit that pair's Q/K proj, 1 kd per kt).
                pg0: pre-loaded x^T group 0 for the proj (or None).
                Returns (normalize closure, pre-emitted next score group,
                pre-loaded group 0 for the NEXT (proj, qc), next-pair u)."""
                u_ret = None
                h0, h1 = 2 * pr, 2 * pr + 1
                jg0 = qc * SC
                cx01 = psum.tile([P, 2 * SC], DT.float32, tag="ps",
                                 name=f"cx{pr}_{qc}")
                if proj is not None:
                    pj_ps = psum.tile([P, 2 * SC], DT.float32, tag="ps",
                                      name=f"pjps{proj}_{qc}")
                    pjq, pjk = pj_ps[:, 0:SC], pj_ps[:, SC:2 * SC]
                    pgs = {0: pg0}

                def emit_proj(kt):
                    g, c = kt // 4, kt % 4
                    if c == 0 and g + 1 < NKG:
                        pgs[g + 1] = load_pgroup(proj, qc, g + 1)
                    kd = kt
                    xt_t = pgs[g][:, c, :]
                    nc.tensor.matmul(
                        pjq, wq_sb[:, kd, proj * P:(proj + 1) * P], xt_t,
                        start=(kd == 0), stop=(kd == NDT - 1),
                    )
                    nc.tensor.matmul(
                        pjk, wk_sb[:, kd, proj * P:(proj + 1) * P], xt_t,
                        start=(kd == 0), stop=(kd == NDT - 1),
                    )

                # 2-deep software pipeline: s(kt+2) is emitted before PV(kt)
                # so the in-order PE queue keeps a backlog (hides LDWEIGHTS
                # and cross-engine semaphore latency).  pending() emits the
                # PREVIOUS qc's deferred normalize chain (DVE+DMA only).
                sq = [s_pre if s_pre is not None else emit_sg(pr, qc, 0),
                      emit_sg(pr, qc, 1)]
                for kt in range(NKT):
                    if kt + 2 < NKT:
                        sq.append(emit_sg(pr, qc, kt + 2))
                    if proj is not None:
                        emit_proj(kt)
                    if kt == 2 and pending is not None:
                        pending[0]()
                    if kt == 8 and upf is not None:
                        u_ret = upf()
                    if kt == 12 and pending is not None:
                        pending[1]()
                    s01 = sq[kt]
                    if kt == 0 and px_pre is not None:
                        px = px_pre
                    else:
                        px = pexpp.tile([P, 2 * SC], DT.bfloat16, tag="pexp",
                                        name=f"px{pr}_{qc}_{kt}")
                        nc.scalar.activation(
                            out=px[:], in_=s01[:], func=AF.Exp,
                            bias=mask_sb[:, kt:kt + 1],
                            scale=1.0 / math.sqrt(D_KV),
                        )
                        j0 = kt * P + jg0
                        nc.vector.tensor_tensor(
                            px.rearrange("p (h j) -> p h j", h=2),
                            px.rearrange("p (h j) -> p h j", h=2),
                            u_t[:, :, j0:j0 + SC], OP.mult
                        )
                    nc.tensor.matmul(
                        cx01[0:65, 0:SC], vaug[:, kt, pr, 0:65], px[:, 0:SC],
                        start=(kt == 0), stop=(kt == NKT - 1),
                    )
                    nc.tensor.matmul(
                        cx01[:, SC:2 * SC], vaug[:, kt, pr, 65:VW],
                        px[:, SC:2 * SC],
                        start=(kt == 0), stop=(kt == NKT - 1),
                    )

                # proj drain (reversed q for qt)
                if proj is not None:
                    nc.scalar.copy(rev_chunk(qt_pq[proj][NQC - 1 - qc][:, :]),
                                   pjq)
                    nc.vector.tensor_copy(kt_pq[proj][qc][:, :], pjk)

                # prefetch group 0 of the NEXT (proj, qc)'s x^T
                pg_next = None
                if qc + 1 < NQC and proj is not None:
                    pg_next = load_pgroup(proj, qc + 1, 0)
                elif qc == NQC - 1 and proj is not None and proj + 1 < NPAIR:
                    pg_next = load_pgroup(proj + 1, 0, 0)

                # pre-emit the NEXT qc's first score group AND its exp +
                # erel multiply, so the boundary Vector burst (proj drain +
                # cx evacuation) never delays the next qc's first PV
                s_next = None
                px_next = None
                if nxt_sg is not None:
                    s_next, npr, nqc, nu = nxt_sg()
                    px_next = pexpp.tile([P, 2 * SC], DT.bfloat16,
                                         tag="pexp", name=f"pxp{npr}_{nqc}")
                    nc.scalar.activation(
                        out=px_next[:], in_=s_next[:], func=AF.Exp,
                        bias=mask_sb[:, 0:1], scale=1.0 / math.sqrt(D_KV),
                    )
                    nc.vector.tensor_tensor(
                        px_next.rearrange("p (h j) -> p h j", h=2),
                        px_next.rearrange("p (h j) -> p h j", h=2),
                        nu[:, :, nqc * SC:nqc * SC + SC], OP.mult
                    )

                # ---- evacuate cx to SBUF (frees the PSUM slot), then the
                # rest of normalize+writeback is DEFERRED into the next qc
                # (DVE + DMA only; the PE never touches it) ----
                cxs = stage.tile([P, 2 * SC], DT.bfloat16, tag="cxs",
                                 name=f"cxs{pr}_{qc}", bufs=1)
                nc.vector.tensor_copy(cxs[:], cx01[:])

                bc_box = {}

                def norm_a():
                    # denominators: h0 on row 64 (cols 0:512), h1 on row 32
                    # (cols 512:1024).  Custom DVE ops need base-partition-0
                    # operands, so pack both rows into a base-0 tile first.
                    # Runs at kt==2 of the NEXT qc; the DRAM-bounce broadcast
                    # DMAs get ~8 k-tiles of latency slack before norm_b's
                    # tensor_tensors (at kt==8) consume bc_sb - the ~5us
                    # chain latency never blocks the Vector queue.
                    dnf = stage.tile([P, SC], DT.float32, tag="dnf",
                                     name=f"dnf{pr}_{qc}", bufs=1)
                    nc.vector.tensor_copy(dnf[64:65, :], cxs[64:65, 0:SC])
                    nc.vector.tensor_copy(dnf[32:33, :], cxs[32:33, SC:2 * SC])
                    rb = stage.tile([P, SC], DT.float32, tag="rb",
                                    name=f"rb{pr}_{qc}", bufs=1)
                    nc.vector.reciprocal_approx_fast(out=rb[:], in_=dnf[:])
                    rbh = stage.tile([P, SC], DT.bfloat16, tag="rbh",
                                     name=f"rbh{pr}_{qc}", bufs=1)
                    nc.vector.tensor_copy(rbh[64:65, :], rb[64:65, :])
                    nc.vector.tensor_copy(rbh[32:33, :], rb[32:33, :])
                    # broadcast across partitions: bounce the reciprocal
                    # rows through DRAM, then stride-0 reads replicate them.
                    # norm_b consumes bc_sb only at kt==12, so the DMA chain
                    # has ~13us of slack before it can block Vector.
                    bnc = dramp.tile([2, SC], DT.bfloat16, tag="bnc",
                                     name=f"bnc{pr}_{qc}")
                    nc.sync.dma_start(bnc[0:1, :], rbh[64:65, :])
                    nc.sync.dma_start(bnc[1:2, :], rbh[32:33, :])
                    bc_sb = stage.tile([P, SC], DT.bfloat16, tag="bc",
                                       name=f"bcs{pr}_{qc}", bufs=2)
                    src0 = bass.AP(tensor=bnc.tensor, offset=bnc.offset,
                                   ap=[[0, 64], [1, SC]])
                    src1 = bass.AP(tensor=bnc.tensor, offset=bnc.offset + SC,
                                   ap=[[0, 64], [1, SC]])
                    nc.gpsimd.dma_start(bc_sb[0:64, :], src0)
                    nc.gpsimd.dma_start(bc_sb[64:128, :], src1)
                    # (writes ride sync/HWDGE: the SWDGE write completion
                    # was taking ~8us under load, starving the read chain)
                    bc_box["bc"] = bc_sb

                def norm_b():
                    bc_sb = bc_box["bc"]
                    ct = ctxt_pq[pr][NQC - 1 - qc]
                    nc.vector.tensor_tensor(
                        rev_chunk(ct[0:64, :]),
                        cxs[0:64, 0:SC], bc_sb[0:64, :], OP.mult)
                    nc.vector.tensor_tensor(
                        rev_chunk(ct[64:128, :]),
                        cxs[64:128, SC:2 * SC], bc_sb[64:128, :], OP.mult)
                return (norm_a, norm_b), s_next, px_next, pg_next, u_ret

            u_t = u0  # pair-0 table already loaded during phase B
            pending = None
            s_pre = None
            px_pre = None
            pg0 = load_pgroup(1, 0, 0)
            seq = [(pr, qc) for pr in range(NPAIR) for qc in range(NQC)]
            next_u = None
            for idx, (pr, qc) in enumerate(seq):
                nxt = pr + 1 if pr + 1 < NPAIR else None
                if nxt is not None and qc == 0:
                    upf = (lambda nxt=nxt: load_u(nxt))
                else:
                    upf = None
                if idx + 1 < len(seq):
                    npr, nqc = seq[idx + 1]
                    # the u table the NEXT (pr,qc) will multiply with: at a
                    # pair boundary it is the freshly loaded next_u
                    nxt_sg = (lambda npr=npr, nqc=nqc:
                              (emit_sg(npr, nqc, 0), npr, nqc,
                               u_t if npr == pr else next_u))
                else:
                    nxt_sg = None
                pending, s_pre, px_pre, pg0, u_ret = attn_qc(
                    pr, qc, u_t, nxt, pending, s_pre, nxt_sg, pg0, upf,
                    px_pre)
                if u_ret is not None:
                    next_u = u_ret
                if qc == NQC - 1 and nxt is not None:
                    u_t = next_u
                    next_u = None
            pending[0]()
            pending[1]()

            # ---------- phase D: output projection (descending st: the
            # low-st tiles depend on the last deferred normalize) ----------
            for st in range(NKT - 1, -1, -1):
                oa = psum.tile([P, 2 * SC], DT.float32, tag="ps",
                               name=f"oa{st}")
                ob = psum.tile([P, 2 * SC], DT.float32, tag="ps",
                               name=f"ob{st}")
                o_ps = [oa[:, 0:SC], oa[:, SC:2 * SC],
                        ob[:, 0:SC], ob[:, SC:2 * SC]]
                for m in range(NMT):
                    for nd in range(NQC):
                        nc.tensor.matmul(
                            o_ps[nd],
                            ctxt_pq[m][st // 4][:, (st % 4) * P:
                                                (st % 4) * P + P],
                            wo_sb[:, m, nd * SC:(nd + 1) * SC],
                            start=(m == 0), stop=(m == NMT - 1),
                        )
                for half in range(2):
                    # separate staging tiles per engine: a shared tile
                    # serialized the vector copy behind the scalar copy
                    o_a = outp.tile([P, SC], DT.float32, tag="outa",
                                    name=f"ota{st}_{half}")
                    o_b = outp.tile([P, SC], DT.float32, tag="outb",
                                    name=f"otb{st}_{half}")
                    nc.scalar.copy(o_a[:], o_ps[2 * half])
                    nc.vector.tensor_copy(o_b[:], o_ps[2 * half + 1])
                    eng = nc.sync if half == 0 else nc.gpsimd
                    c0 = half * 2 * SC
                    eng.dma_start(
                        out[st * P:(st + 1) * P, c0:c0 + SC], o_a[:])
                    eng.dma_start(
                        out[st * P:(st + 1) * P, c0 + SC:c0 + 2 * SC], o_b[:])

    nc.finalize()
    return nc


_NC_CACHE = None


def _get_nc():
    global _NC_CACHE
    if _NC_CACHE is None:
        _NC_CACHE = _build()
    return _NC_CACHE


def _in_maps(hidden_states, attention_mask, Wq, Wk, Wv, Wo, rel_emb):
    import ml_dtypes
    bf16 = ml_dtypes.bfloat16
    maps = []
    for c in range(NCORES):
        b, g = c // 4, c % 4
        hlo, hhi = g * HL, (g + 1) * HL
        erel = _bias_table(
            np.ascontiguousarray(rel_emb[:, hlo:hhi], dtype=np.float32))
        def shuf(w):  # [NDT*P, C] -> [P, NDT*C] partition-contiguous
            cc = w.shape[1]
            return np.ascontiguousarray(
                w.reshape(-1, P, cc).transpose(1, 0, 2).reshape(P, -1))
        # xt[p, qc, kd, j] = x[qc*512+j, kd*128+p]
        xtt = np.ascontiguousarray(
            hidden_states[b].reshape(NQC, SC, NDT, P).transpose(3, 0, 2, 1)
        ).astype(bf16)
        maps.append({
            "xt": xtt,
            "wq": shuf(Wq[:, hlo * D_KV:hhi * D_KV]).astype(bf16),
            "wk": shuf(Wk[:, hlo * D_KV:hhi * D_KV]).astype(bf16),
            "wv": shuf(Wv[:, hlo * D_KV:hhi * D_KV]).astype(bf16),
            "wo": shuf(Wo[hlo * D_KV:hhi * D_KV, :]).astype(bf16),
            "mask": np.ascontiguousarray(attention_mask[b, 0, 0, :]).astype(np.float32),
            "erel": erel,
        })
    return maps


def kernel(hidden_states, attention_mask, Wq, Wk, Wv, Wo, rel_emb, _trace=False,
           _trace_kwargs=None):
    hidden_states = np.asarray(hidden_states, dtype=np.float32)
    attention_mask = np.asarray(attention_mask, dtype=np.float32)
    Wq = np.asarray(Wq, dtype=np.float32)
    Wk = np.asarray(Wk, dtype=np.float32)
    Wv = np.asarray(Wv, dtype=np.float32)
    Wo = np.asarray(Wo, dtype=np.float32)
    rel_emb = np.asarray(rel_emb, dtype=np.float32)

    nc = _get_nc()
    maps = _in_maps(hidden_states, attention_mask, Wq, Wk, Wv, Wo, rel_emb)
    kw = dict(_trace_kwargs or {})
    res = run_bass_kernel_spmd(nc, maps, core_ids=list(range(NCORES)),
                               trace=_trace, **kw)
    kernel.last_results = res
    outp = np.empty((B, S, D), dtype=np.float32)
    for b in range(B):
        acc = np.asarray(res.results[4 * b]["out"], dtype=np.float32).copy()
        for g in range(1, 4):
            acc += np.asarray(res.results[4 * b + g]["out"], dtype=np.float32)
        outp[b] = acc
    return outp


# revision 26
# speedup vs baseline: 1.0434x; 1.0152x over previous
"""T5-style encoder self-attention (dense_transformer) on 8 Trainium2 NeuronCores.

Problem (full shapes): hidden [2,2048,2048], Wq/Wk/Wv/Wo [2048,2048],
rel_emb [32,32] (bidirectional T5 relative-position bias), mask [2,1,1,2048].

Sharding: data-parallel over batch (2) x tensor-parallel over heads (4 groups
of 8 heads) = 8 cores, Megatron-style. Each core computes a partial output
[2048,2048] for its batch (its 8 heads through its Wo row-slice); the host
sums 4 partials per batch.

Per-core kernel design (bf16 operands, fp32 PSUM accumulation):
  - The relative-position bias is applied MULTIPLICATIVELY for every tile:
    px = exp(s/8 + mask) * erel, where erel = exp(bias) is a host-computed
    [8 heads, 4096 diagonals] bf16 table read through a Toeplitz shear view
    (partition stride 1, free stride 1).  Host numerics sim puts the
    all-multiplicative absmax-rel at 7.7e-3 (vs 6.0e-3 for the old scheme
    that identity-injected near-diagonal tiles into PSUM; the inject matmuls
    cost ~38us of PE issue time and a 16us identity DMA on the critical
    startup path).
  - Phase B: single pass over x^T computes pair-0 Q^T/K^T and V for ALL
    heads (6 matmuls per x^T chunk, PE-bound).  Q^T is stored with s
    REVERSED so the bias becomes a positive-shear Toeplitz.
  - Phase C attention, per (head-pair, q-chunk), k-tile loop pipelined one
    iteration ahead:
      * the two per-head QK matmuls are packed as concurrent 64-row-group
        tiles (tile_position (0,0)/(64,0));
      * ACT computes exp(s/8 + mask) in one [128,1024] shot per k-tile;
        DVE multiplies by the erel shear slice (far AND near tiles);
      * next-pair Q/K projection matmuls are interleaved PER k-tile so they
        fill the PE's ACT-wait bubbles; their x^T tiles are group-loaded
        (4 k-chunks per DMA, 4KB partition lines) and prefetched one group
        ahead so the proj matmuls never wait on DMA.
  - x^T is host-tiled to [128, NQC, NDT, 512] so every [128, 4, 512] group
    load has 4KB contiguous per-partition lines (the old [D,S] layout gave
    1KB lines, which capped each DMA queue near 85 GB/s and stalled the
    interleaved projections).
  - V augmentation: per pair, even head block = [v(0:64) | ones(64)] (M=65,
    denominator lands on PSUM partition 64), odd head block = 128 wide with
    ones at col 32 and v at cols 64:128 (denominator on partition 32, ctx on
    partitions 64:128), keeping every normalize op partition-aligned.
  - Normalize is DEFERRED and PE-free: cx evacuates to SBUF at qc end
    (freeing its PSUM slot), then one qc later a DVE+DMA-only chain runs:
    pack denominator rows to a base-0 tile (custom DVE ops require base
    partition 0), reciprocal_approx_fast, bounce the two reciprocal rows
    through DRAM, stride-0 DMAs broadcast them across partitions, and fused
    DVE tensor_tensors do normalize + un-reverse + bf16 writeback.
  - The next qc's first score-group is pre-emitted in the current qc's tail
    (exactly one PSUM slot is free there) so ACT never idles at boundaries.
  - Startup: the first x^T group and the first wq/wk/wv chunk are the FIRST
    DMAs on their queues (sync / gpsimd), so the first matmul fires ~9us in
    instead of ~41us; mask + ACT-table warmup + wo ride the scalar queue.
  - Phase D output projection: descending s-tiles (low tiles depend on the
    last deferred normalize), m looped inside nd so consecutive matmuls hit
    different PSUM banks; evacuation alternates ACT/DVE; the two out DMAs
    per s-tile alternate sync/gpsimd queues.
"""

import math
import sys

for _p in ("/opt/trn_rl_repo",):
    if _p not in sys.path:
        sys.path.insert(0, _p)

import numpy as np

import concourse.bass as bass
import concourse.mybir as mybir
import concourse.tile as tile
from concourse import bacc
from concourse.bass_utils import run_bass_kernel_spmd

DT = mybir.dt
AF = mybir.ActivationFunctionType
OP = mybir.AluOpType

# ---- problem constants (hardcoded per contract) ----
B, S, D = 2, 2048, 2048
N_HEADS, D_KV = 32, 64
NUM_BUCKETS, MAX_DISTANCE = 32, 128
NCORES = 8
HL = 8            # heads per core
P = 128
SC = 512          # free-dim chunk
NKT = S // P      # 16 k-tiles
NQC = S // SC     # 4 q-chunks
NDT = D // P      # 16 D-tiles
NMT = (HL * D_KV) // P   # 4 hd m-tiles per core
NPAIR = HL // 2   # 4 head pairs per core
NDIAG = 4096
W_U = 3968        # erel shear tile width (covers all diagonals any tile hits)
VW = 193          # vaug per-(kt,pair) width: even block 65 + odd block 128
NKG = NDT // 4    # 4 kd-groups of 4 chunks per q-chunk (x^T group loads)


def _rel_bucket_host(d):
    """Exact numpy replica of reference._relative_position_bucket."""
    num_buckets = NUM_BUCKETS // 2          # 16
    max_exact = num_buckets // 2            # 8
    rel = np.asarray(d, dtype=np.int64)
    buckets = (rel > 0).astype(np.int32) * num_buckets
    arel = np.abs(rel)
    is_small = arel < max_exact
    rp_safe = np.maximum(arel, 1).astype(np.float32)
    log_ratio = np.log(rp_safe / np.float32(max_exact)).astype(np.float32)
    scale = np.float32(math.log(MAX_DISTANCE / max_exact))
    rp_large = max_exact + (log_ratio / scale * np.float32(num_buckets - max_exact)).astype(np.int32)
    rp_large = np.minimum(rp_large, num_buckets - 1)
    buckets = buckets + np.where(is_small, arel.astype(np.int32), rp_large)
    return buckets.astype(np.int32)


def _bias_table(rel_emb_slice):
    """rel_emb_slice: [NUM_BUCKETS, HL] fp32 -> erel [HL, NDIAG] bf16,
    erel[h, i] = exp(bias(d = i - 2047)); erel[:, 4095] is never read."""
    import ml_dtypes
    i = np.arange(NDIAG - 1)
    b = _rel_bucket_host(i - (S - 1))                  # [4095]
    vals = rel_emb_slice[b, :]                         # [4095, HL] fp32
    erel = np.zeros((HL, NDIAG), dtype=np.float32)
    erel[:, : NDIAG - 1] = np.exp(vals.T)
    return erel.astype(ml_dtypes.bfloat16)


def _build():
    nc = bacc.Bacc(None, name="attn_tp")

    # x^T host-tiled: xt[p, qc, kd, j] = x[qc*512+j, kd*128+p], so a
    # [128, 4, 512] kd-group load is one DMA with 4KB per-partition lines
    xt = nc.declare_dram_parameter("xt", [P, NQC, NDT, SC], DT.bfloat16,
                                   isOutput=False)
    # weights arrive HOST-SHUFFLED to [p][kt][h] so per-partition lines are
    # contiguous multi-KB runs (DMA packet rate is the limiter at 1KB lines)
    wq = nc.declare_dram_parameter("wq", [P, NDT * HL * D_KV], DT.bfloat16, isOutput=False)
    wk = nc.declare_dram_parameter("wk", [P, NDT * HL * D_KV], DT.bfloat16, isOutput=False)
    wv = nc.declare_dram_parameter("wv", [P, NDT * HL * D_KV], DT.bfloat16, isOutput=False)
    wo = nc.declare_dram_parameter("wo", [P, NMT * D], DT.bfloat16, isOutput=False)
    mask = nc.declare_dram_parameter("mask", [S], DT.float32, isOutput=False)
    erel = nc.declare_dram_parameter("erel", [HL, NDIAG], DT.bfloat16, isOutput=False)
    out = nc.declare_dram_parameter("out", [S, D], DT.float32, isOutput=True)

    with tile.TileContext(nc) as tc:
        with (
            tc.tile_pool(name="res", bufs=1) as res,          # persistent tensors
            tc.tile_pool(name="xtp", bufs=3) as xtp,          # x^T groups (sync q)
            tc.tile_pool(name="upool", bufs=2) as upool,      # exp-bias shear tiles
            tc.tile_pool(name="pexp", bufs=3) as pexpp,       # probs tiles
            tc.tile_pool(name="stage", bufs=2) as stage,      # normalize staging
            tc.tile_pool(name="outp", bufs=3) as outp,        # out staging
            tc.tile_pool(name="psum", bufs=4, space="PSUM") as psum,  # [128,1024] slots
            tc.tile_pool(name="dram", bufs=2, space="DRAM") as dramp,
        ):
            # ---------- constants / resident tensors ----------
            mask_sb = res.tile([P, NKT], DT.float32, tag="mask")
            # mask + ACT exp-table warm-up ride the (otherwise idle) scalar
            # queue so the sync/gpsimd queues start with the critical loads
            nc.scalar.dma_start(mask_sb[:], mask.ap().rearrange("(kt p) -> p kt", p=P))

            wq_sb = res.tile([P, NDT, HL * D_KV], DT.bfloat16, tag="wq")
            wk_sb = res.tile([P, NDT, HL * D_KV], DT.bfloat16, tag="wk")
            wv_sb = res.tile([P, NDT, HL * D_KV], DT.bfloat16, tag="wv")
            wo_sb = res.tile([P, NMT, D], DT.bfloat16, tag="wo")

            # persistent activations.  qt/kt/ctxt are split per (pair,
            # q-chunk) so the tile dep tracker never serializes score /
            # phase-D reads behind a LATER chunk's drain writes (the
            # tracker is not interval-precise across a big tensor).
            qt_pq = [[res.tile([P, SC], DT.bfloat16, tag=f"qt{m}_{c}",
                               name=f"qt{m}_{c}") for c in range(NQC)]
                     for m in range(NMT)]                      # q REVERSED
            kt_pq = [[res.tile([P, SC], DT.bfloat16, tag=f"kt{m}_{c}",
                               name=f"kt{m}_{c}") for c in range(NQC)]
                     for m in range(NMT)]
            vaug = res.tile([P, NKT, NPAIR, VW], DT.bfloat16, tag="vaug")
            ctxt_pq = [[res.tile([P, SC], DT.bfloat16, tag=f"ct{m}_{c}",
                                 name=f"ct{m}_{c}") for c in range(NQC)]
                       for m in range(NMT)]
            # only the two ones-columns are ever read outside the V blocks
            # (psum rows other than the denominator rows are never consumed)
            nc.vector.memset(vaug[:, :, :, 64:65], 1.0)
            nc.vector.memset(vaug[:, :, :, 97:98], 1.0)

            # ACT exp table warm-up (hide the ~2.7us table load under phase B)
            warm = res.tile([1, 2], DT.float32, tag="warm")
            nc.scalar.activation(out=warm[0:1, 0:1], in_=mask_sb[0:1, 0:1], func=AF.Exp)

            def rev_chunk(t):
                """reversed-q view over one [rows, SC] chunk tile: writing
                reversed data lands in natural order.  A logical q-chunk qc
                written reversed covers NATURAL chunk NQC-1-qc."""
                return bass.AP(
                    tensor=t.tensor,
                    offset=t.offset + (SC - 1),
                    ap=[list(t.ap[0]), [-1, SC]],
                )

            def load_u(pr, eng=None):
                """erel shear tile [P, 2, W_U] for pair pr: u[p, i, w] =
                erel[2*pr+i, p + w].  One DMA per head: fewer, larger DMAs
                keep the issuing queue free (the scheduler bunches split
                loads into queue-hogging bursts anyway)."""
                u = upool.tile([P, 2, W_U], DT.bfloat16, tag="u",
                               name=f"u{pr}", bufs=2)
                ap0 = erel.ap()
                for i, hh in enumerate((2 * pr, 2 * pr + 1)):
                    shear = bass.AP(
                        tensor=ap0.tensor,
                        offset=ap0.offset + hh * NDIAG,
                        ap=[[1, P], [1, W_U]],
                    )
                    (eng or nc.gpsimd).dma_start(u[:, i, :], shear)
                return u

            def load_wchunk(g, eng=None):
                """one 4-kd chunk of weights; chunk 0 rides sync (HWDGE,
                ~0.6us first byte) right behind the first x group so the
                first matmuls fire ~10us in; later chunks stream on gpsimd
                in kd order."""
                eng = eng or nc.gpsimd
                cw = HL * D_KV
                c0, c1 = g * 4 * cw, (g + 1) * 4 * cw
                eng.dma_start(wq_sb[:, g * 4:(g + 1) * 4, :], wq[:, c0:c1])
                eng.dma_start(wk_sb[:, g * 4:(g + 1) * 4, :], wk[:, c0:c1])
                eng.dma_start(wv_sb[:, g * 4:(g + 1) * 4, :], wv[:, c0:c1])

            def load_xgroup(nq, g):
                """one [128, 4, 512] x^T kd-group (4KB partition lines)."""
                t = xtp.tile([P, 4, SC], DT.bfloat16, tag="xt",
                             name=f"xg{nq}_{g}")
                nc.sync.dma_start(t[:], xt[:, nq, 4 * g:4 * (g + 1), :])
                return t

            # ---------- phase B: pair-0 Q/K + V (all heads), single x^T pass ----
            wc0_loaded = False
            for nq in range(NQC):
                qk_ps = psum.tile([P, 2 * SC], DT.float32, tag="ps",
                                  name=f"qkps0_{nq}")
                q_ps, k_ps = qk_ps[:, 0:SC], qk_ps[:, SC:2 * SC]
                v01 = psum.tile([P, 2 * SC], DT.float32, tag="ps", name=f"v01_{nq}")
                v23 = psum.tile([P, 2 * SC], DT.float32, tag="ps", name=f"v23_{nq}")
                v_ps = [v01[:, 0:SC], v01[:, SC:2 * SC],
                        v23[:, 0:SC], v23[:, SC:2 * SC]]
                for g in range(NKG):
                    xg = load_xgroup(nq, g)
                    if nq == 0 and not wc0_loaded:
                        load_wchunk(0, eng=nc.sync)
                        wc0_loaded = True
                    if nq == 0 and g + 1 < NKG:
                        load_wchunk(g + 1)   # prefetch next weight chunk
                    for c in range(4):
                        kd = 4 * g + c
                        xt_t = xg[:, c, :]
                        nc.tensor.matmul(
                            q_ps, wq_sb[:, kd, 0:P], xt_t,
                            start=(kd == 0), stop=(kd == NDT - 1),
                        )
                        nc.tensor.matmul(
                            k_ps, wk_sb[:, kd, 0:P], xt_t,
                            start=(kd == 0), stop=(kd == NDT - 1),
                        )
                        for st in range(4):
                            nc.tensor.matmul(
                                v_ps[st], xg[:, c, st * P:(st + 1) * P],
                                wv_sb[:, kd, :],
                                start=(kd == 0), stop=(kd == NDT - 1),
                            )
                if nq == 0:
                    # pair-0 u table behind the weight chunks on gpsimd
                    # (phase C only needs it ~90us in)
                    u0 = load_u(0)
                # drain: V -> vaug blocks first (frees the 2 V psum slots the
                # next nq's V matmuls are waiting on), then q/k casts
                for st in range(4):
                    ktg = nq * 4 + st
                    vsrc = v_ps[st].rearrange("p (pr par d) -> p pr par d",
                                              par=2, d=D_KV)
                    nc.vector.tensor_copy(vaug[:, ktg, :, 0:D_KV],
                                          vsrc[:, :, 0, :])
                    nc.vector.tensor_copy(vaug[:, ktg, :, 129:193],
                                          vsrc[:, :, 1, :])
                nc.vector.tensor_copy(rev_chunk(qt_pq[0][NQC - 1 - nq][:, :]),
                                      q_ps)
                nc.vector.tensor_copy(kt_pq[0][nq][:, :], k_ps)

            # wo load issues from the idle scalar queue once phase B's
            # critical streams are done (needed only in phase D)
            nc.scalar.dma_start(wo_sb.rearrange("p a b -> p (a b)"), wo[:])

            # ---------- phase C: attention, proj of pair pr+1 interleaved ----
            def emit_sg(pr, qc, kt):
                """scores psum group for (pair, q-chunk, k-tile): the two
                heads run as concurrent 64-row-group tiles."""
                jg0 = qc * SC
                s01 = psum.tile([P, 2 * SC], DT.float32, tag="ps",
                                name=f"s{pr}_{qc}_{kt}")
                kc, ko = kt // 4, (kt % 4) * P
                nc.tensor.matmul(
                    s01[:, 0:SC], kt_pq[pr][kc][0:64, ko:ko + P],
                    qt_pq[pr][qc][0:64, :],
                    start=True, stop=True, tile_position=(0, 0),
                )
                nc.tensor.matmul(
                    s01[:, SC:2 * SC], kt_pq[pr][kc][64:128, ko:ko + P],
                    qt_pq[pr][qc][64:128, :],
                    start=True, stop=True, tile_position=(64, 0),
                )
                return s01

            # proj x^T group tiles, prefetched one group ahead (keyed by
            # group index within the current (proj, qc))
            def load_pgroup(proj, qc, g):
                t = xtp.tile([P, 4, SC], DT.bfloat16, tag="xt",
                             name=f"xp{proj}_{qc}_{g}")
                nc.sync.dma_start(t[:], xt[:, qc, 4 * g:4 * (g + 1), :])
                return t

            def attn_qc(pr, qc, u_t, proj, pending, s_pre, nxt_sg, pg0,
                        upf=None, px_pre=None):
                """attention for head pair pr, reversed-q chunk qc.
                proj: None or pr+1 (emit that pair's Q/K proj, 1 kd per kt).
                pg0: pre-loaded x^T group 0 for the proj (or None).
                Returns (normalize closure, pre-emitted next score group,
                pre-loaded group 0 for the NEXT (proj, qc), next-pair u)."""
                u_ret = None
                h0, h1 = 2 * pr, 2 * pr + 1
                jg0 = qc * SC
                cx01 = psum.tile([P, 2 * SC], DT.float32, tag="ps",
                                 name=f"cx{pr}_{qc}")
                if proj is not None:
                    pj_ps = psum.tile([P, 2 * SC], DT.float32, tag="ps",
                                      name=f"pjps{proj}_{qc}")
                    pjq, pjk = pj_ps[:, 0:SC], pj_ps[:, SC:2 * SC]
                    pgs = {0: pg0}

                def emit_proj(kt):
                    g, c = kt // 4, kt % 4
                    if c == 0 and g + 1 < NKG:
                        pgs[g + 1] = load_pgroup(proj, qc, g + 1)
                    kd = kt
                    xt_t = pgs[g][:, c, :]
                    nc.tensor.matmul(
                        pjq, wq_sb[:, kd, proj * P:(proj + 1) * P], xt_t,
                        start=(kd == 0), stop=(kd == NDT - 1),
                    )
                    nc.tensor.matmul(
                        pjk, wk_sb[:, kd, proj * P:(proj + 1) * P], xt_t,
                        start=(kd == 0), stop=(kd == NDT - 1),
                    )

                # 2-deep software pipeline: s(kt+2) is emitted before PV(kt)
                # so the in-order PE queue keeps a backlog (hides LDWEIGHTS
                # and cross-engine semaphore latency).  pending() emits the
                # PREVIOUS qc's deferred normalize chain (DVE+DMA only).
                sq = [s_pre if s_pre is not None else emit_sg(pr, qc, 0),
                      emit_sg(pr, qc, 1)]
                for kt in range(NKT):
                    if kt + 2 < NKT:
                        sq.append(emit_sg(pr, qc, kt + 2))
                    if proj is not None:
                        emit_proj(kt)
                    if kt == 2 and pending is not None:
                        pending[0]()
                    if kt == 8 and upf is not None:
                        u_ret = upf()
                    if kt == 12 and pending is not None:
                        pending[1]()
                    s01 = sq[kt]
                    if kt == 0 and px_pre is not None:
                        px = px_pre
                    else:
                        px = pexpp.tile([P, 2 * SC], DT.bfloat16, tag="pexp",
                                        name=f"px{pr}_{qc}_{kt}")
                        nc.scalar.activation(
                            out=px[:], in_=s01[:], func=AF.Exp,
                            bias=mask_sb[:, kt:kt + 1],
                            scale=1.0 / math.sqrt(D_KV),
                        )
                        j0 = kt * P + jg0
                        nc.vector.tensor_tensor(
                            px.rearrange("p (h j) -> p h j", h=2),
                            px.rearrange("p (h j) -> p h j", h=2),
                            u_t[:, :, j0:j0 + SC], OP.mult
                        )
                    nc.tensor.matmul(
                        cx01[0:65, 0:SC], vaug[:, kt, pr, 0:65], px[:, 0:SC],
                        start=(kt == 0), stop=(kt == NKT - 1),
                    )
                    nc.tensor.matmul(
                        cx01[:, SC:2 * SC], vaug[:, kt, pr, 65:VW],
                        px[:, SC:2 * SC],
                        start=(kt == 0), stop=(kt == NKT - 1),
                    )

                # proj drain (reversed q for qt)
                if proj is not None:
                    nc.scalar.copy(rev_chunk(qt_pq[proj][NQC - 1 - qc][:, :]),
                                   pjq)
                    nc.vector.tensor_copy(kt_pq[proj][qc][:, :], pjk)

                # prefetch group 0 of the NEXT (proj, qc)'s x^T
                pg_next = None
                if qc + 1 < NQC and proj is not None:
                    pg_next = load_pgroup(proj, qc + 1, 0)
                elif qc == NQC - 1 and proj is not None and proj + 1 < NPAIR:
                    pg_next = load_pgroup(proj + 1, 0, 0)

                # pre-emit the NEXT qc's first score group AND its exp +
                # erel multiply, so the boundary Vector burst (proj drain +
                # cx evacuation) never delays the next qc's first PV
                s_next = None
                px_next = None
                if nxt_sg is not None:
                    s_next, npr, nqc, nu = nxt_sg()
                    px_next = pexpp.tile([P, 2 * SC], DT.bfloat16,
                                         tag="pexp", name=f"pxp{npr}_{nqc}")
                    nc.scalar.activation(
                        out=px_next[:], in_=s_next[:], func=AF.Exp,
                        bias=mask_sb[:, 0:1], scale=1.0 / math.sqrt(D_KV),
                    )
                    nc.vector.tensor_tensor(
                        px_next.rearrange("p (h j) -> p h j", h=2),
                        px_next.rearrange("p (h j) -> p h j", h=2),
                        nu[:, :, nqc * SC:nqc * SC + SC], OP.mult
                    )

                # ---- evacuate cx to SBUF (frees the PSUM slot), then the
                # rest of normalize+writeback is DEFERRED into the next qc
                # (DVE + DMA only; the PE never touches it) ----
                cxs = stage.tile([P, 2 * SC], DT.bfloat16, tag="cxs",
                                 name=f"cxs{pr}_{qc}", bufs=1)
                nc.vector.tensor_copy(cxs[:], cx01[:])

                bc_box = {}

                def norm_a():
                    # denominators: h0 on row 64 (cols 0:512), h1 on row 32
                    # (cols 512:1024).  Custom DVE ops need base-partition-0
                    # operands, so pack both rows into a base-0 tile first.
                    # Runs at kt==2 of the NEXT qc; the DRAM-bounce broadcast
                    # DMAs get ~8 k-tiles of latency slack before norm_b's
                    # tensor_tensors (at kt==8) consume bc_sb - the ~5us
                    # chain latency never blocks the Vector queue.
                    dnf = stage.tile([P, SC], DT.float32, tag="dnf",
                                     name=f"dnf{pr}_{qc}", bufs=1)
                    nc.vector.tensor_copy(dnf[64:65, :], cxs[64:65, 0:SC])
                    nc.vector.tensor_copy(dnf[32:33, :], cxs[32:33, SC:2 * SC])
                    rb = stage.tile([P, SC], DT.float32, tag="rb",
                                    name=f"rb{pr}_{qc}", bufs=1)
                    nc.vector.reciprocal_approx_fast(out=rb[:], in_=dnf[:])
                    rbh = stage.tile([P, SC], DT.bfloat16, tag="rbh",
                                     name=f"rbh{pr}_{qc}", bufs=1)
                    nc.vector.tensor_copy(rbh[64:65, :], rb[64:65, :])
                    nc.vector.tensor_copy(rbh[32:33, :], rb[32:33, :])
                    # broadcast across partitions: bounce the reciprocal
                    # rows through DRAM, then stride-0 reads replicate them.
                    # norm_b consumes bc_sb only at kt==12, so the DMA chain
                    # has ~13us of slack before it can block Vector.
                    bnc = dramp.tile([2, SC], DT.bfloat16, tag="bnc",
                                     name=f"bnc{pr}_{qc}")
                    nc.sync.dma_start(bnc[0:1, :], rbh[64:65, :])
                    nc.sync.dma_start(bnc[1:2, :], rbh[32:33, :])
                    bc_sb = stage.tile([P, SC], DT.bfloat16, tag="bc",
                                       name=f"bcs{pr}_{qc}", bufs=2)
                    src0 = bass.AP(tensor=bnc.tensor, offset=bnc.offset,
                                   ap=[[0, 64], [1, SC]])
                    src1 = bass.AP(tensor=bnc.tensor, offset=bnc.offset + SC,
                                   ap=[[0, 64], [1, SC]])
                    nc.sync.dma_start(bc_sb[0:64, :], src0)
                    nc.sync.dma_start(bc_sb[64:128, :], src1)
                    # (whole bounce rides sync/HWDGE: SWDGE completions
                    # took ~5-8us under load behind the 2MB u-table loads,
                    # starving norm_b and head-of-line blocking Vector)
                    bc_box["bc"] = bc_sb

                def norm_b():
                    bc_sb = bc_box["bc"]
                    ct = ctxt_pq[pr][NQC - 1 - qc]
                    nc.vector.tensor_tensor(
                        rev_chunk(ct[0:64, :]),
                        cxs[0:64, 0:SC], bc_sb[0:64, :], OP.mult)
                    nc.vector.tensor_tensor(
                        rev_chunk(ct[64:128, :]),
                        cxs[64:128, SC:2 * SC], bc_sb[64:128, :], OP.mult)
                return (norm_a, norm_b), s_next, px_next, pg_next, u_ret

            u_t = u0  # pair-0 table already loaded during phase B
            pending = None
            s_pre = None
            px_pre = None
            pg0 = load_pgroup(1, 0, 0)
            seq = [(pr, qc) for pr in range(NPAIR) for qc in range(NQC)]
            next_u = None
            for idx, (pr, qc) in enumerate(seq):
                nxt = pr + 1 if pr + 1 < NPAIR else None
                if nxt is not None and qc == 0:
                    upf = (lambda nxt=nxt: load_u(nxt))
                else:
                    upf = None
                if idx + 1 < len(seq):
                    npr, nqc = seq[idx + 1]
                    # the u table the NEXT (pr,qc) will multiply with: at a
                    # pair boundary it is the freshly loaded next_u
                    nxt_sg = (lambda npr=npr, nqc=nqc:
                              (emit_sg(npr, nqc, 0), npr, nqc,
                               u_t if npr == pr else next_u))
                else:
                    nxt_sg = None
                pending, s_pre, px_pre, pg0, u_ret = attn_qc(
                    pr, qc, u_t, nxt, pending, s_pre, nxt_sg, pg0, upf,
                    px_pre)
                if u_ret is not None:
                    next_u = u_ret
                if qc == NQC - 1 and nxt is not None:
                    u_t = next_u
                    next_u = None
            pending[0]()
            pending[1]()

            # ---------- phase D: output projection (descending st: the
            # low-st tiles depend on the last deferred normalize) ----------
            for st in range(NKT - 1, -1, -1):
                oa = psum.tile([P, 2 * SC], DT.float32, tag="ps",
                               name=f"oa{st}")
                ob = psum.tile([P, 2 * SC], DT.float32, tag="ps",
                               name=f"ob{st}")
                o_ps = [oa[:, 0:SC], oa[:, SC:2 * SC],
                        ob[:, 0:SC], ob[:, SC:2 * SC]]
                for m in range(NMT):
                    for nd in range(NQC):
                        nc.tensor.matmul(
                            o_ps[nd],
                            ctxt_pq[m][st // 4][:, (st % 4) * P:
                                                (st % 4) * P + P],
                            wo_sb[:, m, nd * SC:(nd + 1) * SC],
                            start=(m == 0), stop=(m == NMT - 1),
                        )
                for half in range(2):
                    # separate staging tiles per engine: a shared tile
                    # serialized the vector copy behind the scalar copy
                    o_a = outp.tile([P, SC], DT.float32, tag="outa",
                                    name=f"ota{st}_{half}")
                    o_b = outp.tile([P, SC], DT.float32, tag="outb",
                                    name=f"otb{st}_{half}")
                    nc.scalar.copy(o_a[:], o_ps[2 * half])
                    nc.vector.tensor_copy(o_b[:], o_ps[2 * half + 1])
                    eng = nc.sync if half == 0 else nc.gpsimd
                    c0 = half * 2 * SC
                    eng.dma_start(
                        out[st * P:(st + 1) * P, c0:c0 + SC], o_a[:])
                    eng.dma_start(
                        out[st * P:(st + 1) * P, c0 + SC:c0 + 2 * SC], o_b[:])

    nc.finalize()
    return nc


_NC_CACHE = None


def _get_nc():
    global _NC_CACHE
    if _NC_CACHE is None:
        _NC_CACHE = _build()
    return _NC_CACHE


def _in_maps(hidden_states, attention_mask, Wq, Wk, Wv, Wo, rel_emb):
    import ml_dtypes
    bf16 = ml_dtypes.bfloat16
    maps = []
    for c in range(NCORES):
        b, g = c // 4, c % 4
        hlo, hhi = g * HL, (g + 1) * HL
        erel = _bias_table(
            np.ascontiguousarray(rel_emb[:, hlo:hhi], dtype=np.float32))
        def shuf(w):  # [NDT*P, C] -> [P, NDT*C] partition-contiguous
            cc = w.shape[1]
            return np.ascontiguousarray(
                w.reshape(-1, P, cc).transpose(1, 0, 2).reshape(P, -1))
        # xt[p, qc, kd, j] = x[qc*512+j, kd*128+p]
        xtt = np.ascontiguousarray(
            hidden_states[b].reshape(NQC, SC, NDT, P).transpose(3, 0, 2, 1)
        ).astype(bf16)
        maps.append({
            "xt": xtt,
            "wq": shuf(Wq[:, hlo * D_KV:hhi * D_KV]).astype(bf16),
            "wk": shuf(Wk[:, hlo * D_KV:hhi * D_KV]).astype(bf16),
            "wv": shuf(Wv[:, hlo * D_KV:hhi * D_KV]).astype(bf16),
            "wo": shuf(Wo[hlo * D_KV:hhi * D_KV, :]).astype(bf16),
            "mask": np.ascontiguousarray(attention_mask[b, 0, 0, :]).astype(np.float32),
            "erel": erel,
        })
    return maps


def kernel(hidden_states, attention_mask, Wq, Wk, Wv, Wo, rel_emb, _trace=False,
           _trace_kwargs=None):
    hidden_states = np.asarray(hidden_states, dtype=np.float32)
    attention_mask = np.asarray(attention_mask, dtype=np.float32)
    Wq = np.asarray(Wq, dtype=np.float32)
    Wk = np.asarray(Wk, dtype=np.float32)
    Wv = np.asarray(Wv, dtype=np.float32)
    Wo = np.asarray(Wo, dtype=np.float32)
    rel_emb = np.asarray(rel_emb, dtype=np.float32)

    nc = _get_nc()
    maps = _in_maps(hidden_states, attention_mask, Wq, Wk, Wv, Wo, rel_emb)
    kw = dict(_trace_kwargs or {})
    res = run_bass_kernel_spmd(nc, maps, core_ids=list(range(NCORES)),
                               trace=_trace, **kw)
    kernel.last_results = res
    outp = np.empty((B, S, D), dtype=np.float32)
    for b in range(B):
        acc = np.asarray(res.results[4 * b]["out"], dtype=np.float32).copy()
        for g in range(1, 4):
            acc += np.asarray(res.results[4 * b + g]["out"], dtype=np.float32)
        outp[b] = acc
    return outp


# revision 27
# speedup vs baseline: 1.0491x; 1.0055x over previous
"""T5-style encoder self-attention (dense_transformer) on 8 Trainium2 NeuronCores.

Problem (full shapes): hidden [2,2048,2048], Wq/Wk/Wv/Wo [2048,2048],
rel_emb [32,32] (bidirectional T5 relative-position bias), mask [2,1,1,2048].

Sharding: data-parallel over batch (2) x tensor-parallel over heads (4 groups
of 8 heads) = 8 cores, Megatron-style. Each core computes a partial output
[2048,2048] for its batch (its 8 heads through its Wo row-slice); the host
sums 4 partials per batch.

Per-core kernel design (bf16 operands, fp32 PSUM accumulation):
  - The relative-position bias is applied MULTIPLICATIVELY for every tile:
    px = exp(s/8 + mask) * erel, where erel = exp(bias) is a host-computed
    [8 heads, 4096 diagonals] bf16 table read through a Toeplitz shear view
    (partition stride 1, free stride 1).  Host numerics sim puts the
    all-multiplicative absmax-rel at 7.7e-3 (vs 6.0e-3 for the old scheme
    that identity-injected near-diagonal tiles into PSUM; the inject matmuls
    cost ~38us of PE issue time and a 16us identity DMA on the critical
    startup path).
  - Phase B: single pass over x^T computes pair-0 Q^T/K^T and V for ALL
    heads (6 matmuls per x^T chunk, PE-bound).  Q^T is stored with s
    REVERSED so the bias becomes a positive-shear Toeplitz.
  - Phase C attention, per (head-pair, q-chunk), k-tile loop pipelined one
    iteration ahead:
      * the two per-head QK matmuls are packed as concurrent 64-row-group
        tiles (tile_position (0,0)/(64,0));
      * ACT computes exp(s/8 + mask) in one [128,1024] shot per k-tile;
        DVE multiplies by the erel shear slice (far AND near tiles);
      * next-pair Q/K projection matmuls are interleaved PER k-tile so they
        fill the PE's ACT-wait bubbles; their x^T tiles are group-loaded
        (4 k-chunks per DMA, 4KB partition lines) and prefetched one group
        ahead so the proj matmuls never wait on DMA.
  - x^T is host-tiled to [128, NQC, NDT, 512] so every [128, 4, 512] group
    load has 4KB contiguous per-partition lines (the old [D,S] layout gave
    1KB lines, which capped each DMA queue near 85 GB/s and stalled the
    interleaved projections).
  - V augmentation: per pair, even head block = [v(0:64) | ones(64)] (M=65,
    denominator lands on PSUM partition 64), odd head block = 128 wide with
    ones at col 32 and v at cols 64:128 (denominator on partition 32, ctx on
    partitions 64:128), keeping every normalize op partition-aligned.
  - Normalize is DEFERRED and PE-free: cx evacuates to SBUF at qc end
    (freeing its PSUM slot), then one qc later a DVE+DMA-only chain runs:
    pack denominator rows to a base-0 tile (custom DVE ops require base
    partition 0), reciprocal_approx_fast, bounce the two reciprocal rows
    through DRAM, stride-0 DMAs broadcast them across partitions, and fused
    DVE tensor_tensors do normalize + un-reverse + bf16 writeback.
  - The next qc's first score-group is pre-emitted in the current qc's tail
    (exactly one PSUM slot is free there) so ACT never idles at boundaries.
  - Startup: the first x^T group and the first wq/wk/wv chunk are the FIRST
    DMAs on their queues (sync / gpsimd), so the first matmul fires ~9us in
    instead of ~41us; mask + ACT-table warmup + wo ride the scalar queue.
  - Phase D output projection: descending s-tiles (low tiles depend on the
    last deferred normalize), m looped inside nd so consecutive matmuls hit
    different PSUM banks; evacuation alternates ACT/DVE; the two out DMAs
    per s-tile alternate sync/gpsimd queues.
"""

import math
import sys

for _p in ("/opt/trn_rl_repo",):
    if _p not in sys.path:
        sys.path.insert(0, _p)

import numpy as np

import concourse.bass as bass
import concourse.mybir as mybir
import concourse.tile as tile
from concourse import bacc
from concourse.bass_utils import run_bass_kernel_spmd

DT = mybir.dt
AF = mybir.ActivationFunctionType
OP = mybir.AluOpType

# ---- problem constants (hardcoded per contract) ----
B, S, D = 2, 2048, 2048
N_HEADS, D_KV = 32, 64
NUM_BUCKETS, MAX_DISTANCE = 32, 128
NCORES = 8
HL = 8            # heads per core
P = 128
SC = 512          # free-dim chunk
NKT = S // P      # 16 k-tiles
NQC = S // SC     # 4 q-chunks
NDT = D // P      # 16 D-tiles
NMT = (HL * D_KV) // P   # 4 hd m-tiles per core
NPAIR = HL // 2   # 4 head pairs per core
NDIAG = 4096
W_U = 3968        # erel shear tile width (covers all diagonals any tile hits)
VW = 193          # vaug per-(kt,pair) width: even block 65 + odd block 128
NKG = NDT // 4    # 4 kd-groups of 4 chunks per q-chunk (x^T group loads)


def _rel_bucket_host(d):
    """Exact numpy replica of reference._relative_position_bucket."""
    num_buckets = NUM_BUCKETS // 2          # 16
    max_exact = num_buckets // 2            # 8
    rel = np.asarray(d, dtype=np.int64)
    buckets = (rel > 0).astype(np.int32) * num_buckets
    arel = np.abs(rel)
    is_small = arel < max_exact
    rp_safe = np.maximum(arel, 1).astype(np.float32)
    log_ratio = np.log(rp_safe / np.float32(max_exact)).astype(np.float32)
    scale = np.float32(math.log(MAX_DISTANCE / max_exact))
    rp_large = max_exact + (log_ratio / scale * np.float32(num_buckets - max_exact)).astype(np.int32)
    rp_large = np.minimum(rp_large, num_buckets - 1)
    buckets = buckets + np.where(is_small, arel.astype(np.int32), rp_large)
    return buckets.astype(np.int32)


def _bias_table(rel_emb_slice):
    """rel_emb_slice: [NUM_BUCKETS, HL] fp32 -> erel [HL, NDIAG] bf16,
    erel[h, i] = exp(bias(d = i - 2047)); erel[:, 4095] is never read."""
    import ml_dtypes
    i = np.arange(NDIAG - 1)
    b = _rel_bucket_host(i - (S - 1))                  # [4095]
    vals = rel_emb_slice[b, :]                         # [4095, HL] fp32
    erel = np.zeros((HL, NDIAG), dtype=np.float32)
    erel[:, : NDIAG - 1] = np.exp(vals.T)
    return erel.astype(ml_dtypes.bfloat16)


def _build():
    nc = bacc.Bacc(None, name="attn_tp")

    # x^T host-tiled: xt[p, qc, kd, j] = x[qc*512+j, kd*128+p], so a
    # [128, 4, 512] kd-group load is one DMA with 4KB per-partition lines
    xt = nc.declare_dram_parameter("xt", [P, NQC, NDT, SC], DT.bfloat16,
                                   isOutput=False)
    # weights arrive HOST-SHUFFLED to [p][kt][h] so per-partition lines are
    # contiguous multi-KB runs (DMA packet rate is the limiter at 1KB lines)
    wq = nc.declare_dram_parameter("wq", [P, NDT * HL * D_KV], DT.bfloat16, isOutput=False)
    wk = nc.declare_dram_parameter("wk", [P, NDT * HL * D_KV], DT.bfloat16, isOutput=False)
    wv = nc.declare_dram_parameter("wv", [P, NDT * HL * D_KV], DT.bfloat16, isOutput=False)
    wo = nc.declare_dram_parameter("wo", [P, NMT * D], DT.bfloat16, isOutput=False)
    mask = nc.declare_dram_parameter("mask", [S], DT.float32, isOutput=False)
    erel = nc.declare_dram_parameter("erel", [HL, NDIAG], DT.bfloat16, isOutput=False)
    out = nc.declare_dram_parameter("out", [S, D], DT.float32, isOutput=True)

    with tile.TileContext(nc) as tc:
        with (
            tc.tile_pool(name="res", bufs=1) as res,          # persistent tensors
            tc.tile_pool(name="xtp", bufs=3) as xtp,          # x^T groups (sync q)
            tc.tile_pool(name="upool", bufs=2) as upool,      # exp-bias shear tiles
            tc.tile_pool(name="pexp", bufs=3) as pexpp,       # probs tiles
            tc.tile_pool(name="stage", bufs=2) as stage,      # normalize staging
            tc.tile_pool(name="outp", bufs=3) as outp,        # out staging
            tc.tile_pool(name="psum", bufs=4, space="PSUM") as psum,  # [128,1024] slots
            tc.tile_pool(name="dram", bufs=2, space="DRAM") as dramp,
        ):
            # ---------- constants / resident tensors ----------
            mask_sb = res.tile([P, NKT], DT.float32, tag="mask")
            # mask + ACT exp-table warm-up ride the (otherwise idle) scalar
            # queue so the sync/gpsimd queues start with the critical loads
            nc.scalar.dma_start(mask_sb[:], mask.ap().rearrange("(kt p) -> p kt", p=P))

            wq_sb = res.tile([P, NDT, HL * D_KV], DT.bfloat16, tag="wq")
            wk_sb = res.tile([P, NDT, HL * D_KV], DT.bfloat16, tag="wk")
            wv_sb = res.tile([P, NDT, HL * D_KV], DT.bfloat16, tag="wv")
            wo_sb = res.tile([P, NMT, D], DT.bfloat16, tag="wo")

            # persistent activations.  qt/kt/ctxt are split per (pair,
            # q-chunk) so the tile dep tracker never serializes score /
            # phase-D reads behind a LATER chunk's drain writes (the
            # tracker is not interval-precise across a big tensor).
            qt_pq = [[res.tile([P, SC], DT.bfloat16, tag=f"qt{m}_{c}",
                               name=f"qt{m}_{c}") for c in range(NQC)]
                     for m in range(NMT)]                      # q REVERSED
            kt_pq = [[res.tile([P, SC], DT.bfloat16, tag=f"kt{m}_{c}",
                               name=f"kt{m}_{c}") for c in range(NQC)]
                     for m in range(NMT)]
            vaug = res.tile([P, NKT, NPAIR, VW], DT.bfloat16, tag="vaug")
            ctxt_pq = [[res.tile([P, SC], DT.bfloat16, tag=f"ct{m}_{c}",
                                 name=f"ct{m}_{c}") for c in range(NQC)]
                       for m in range(NMT)]
            # only the two ones-columns are ever read outside the V blocks
            # (psum rows other than the denominator rows are never consumed)
            nc.vector.memset(vaug[:, :, :, 64:65], 1.0)
            nc.vector.memset(vaug[:, :, :, 97:98], 1.0)

            # ACT exp table warm-up (hide the ~2.7us table load under phase B)
            warm = res.tile([1, 2], DT.float32, tag="warm")
            nc.scalar.activation(out=warm[0:1, 0:1], in_=mask_sb[0:1, 0:1], func=AF.Exp)

            def rev_chunk(t):
                """reversed-q view over one [rows, SC] chunk tile: writing
                reversed data lands in natural order.  A logical q-chunk qc
                written reversed covers NATURAL chunk NQC-1-qc."""
                return bass.AP(
                    tensor=t.tensor,
                    offset=t.offset + (SC - 1),
                    ap=[list(t.ap[0]), [-1, SC]],
                )

            def load_u(pr, eng=None):
                """erel shear tile [P, 2, W_U] for pair pr: u[p, i, w] =
                erel[2*pr+i, p + w].  One DMA per head: fewer, larger DMAs
                keep the issuing queue free (the scheduler bunches split
                loads into queue-hogging bursts anyway)."""
                u = upool.tile([P, 2, W_U], DT.bfloat16, tag="u",
                               name=f"u{pr}", bufs=2)
                ap0 = erel.ap()
                for i, hh in enumerate((2 * pr, 2 * pr + 1)):
                    shear = bass.AP(
                        tensor=ap0.tensor,
                        offset=ap0.offset + hh * NDIAG,
                        ap=[[1, P], [1, W_U]],
                    )
                    (eng or nc.gpsimd).dma_start(u[:, i, :], shear)
                return u

            def load_wchunk(g, eng=None):
                """one 4-kd chunk of weights; chunk 0 rides sync (HWDGE,
                ~0.6us first byte) right behind the first x group so the
                first matmuls fire ~10us in; later chunks stream on gpsimd
                in kd order."""
                eng = eng or nc.gpsimd
                cw = HL * D_KV
                c0, c1 = g * 4 * cw, (g + 1) * 4 * cw
                eng.dma_start(wq_sb[:, g * 4:(g + 1) * 4, :], wq[:, c0:c1])
                eng.dma_start(wk_sb[:, g * 4:(g + 1) * 4, :], wk[:, c0:c1])
                eng.dma_start(wv_sb[:, g * 4:(g + 1) * 4, :], wv[:, c0:c1])

            def load_xgroup(nq, g):
                """one [128, 4, 512] x^T kd-group (4KB partition lines)."""
                t = xtp.tile([P, 4, SC], DT.bfloat16, tag="xt",
                             name=f"xg{nq}_{g}")
                nc.sync.dma_start(t[:], xt[:, nq, 4 * g:4 * (g + 1), :])
                return t

            # ---------- phase B: pair-0 Q/K + V (all heads), single x^T pass ----
            wc0_loaded = False
            for nq in range(NQC):
                qk_ps = psum.tile([P, 2 * SC], DT.float32, tag="ps",
                                  name=f"qkps0_{nq}")
                q_ps, k_ps = qk_ps[:, 0:SC], qk_ps[:, SC:2 * SC]
                v01 = psum.tile([P, 2 * SC], DT.float32, tag="ps", name=f"v01_{nq}")
                v23 = psum.tile([P, 2 * SC], DT.float32, tag="ps", name=f"v23_{nq}")
                v_ps = [v01[:, 0:SC], v01[:, SC:2 * SC],
                        v23[:, 0:SC], v23[:, SC:2 * SC]]
                for g in range(NKG):
                    xg = load_xgroup(nq, g)
                    if nq == 0 and not wc0_loaded:
                        load_wchunk(0, eng=nc.sync)
                        load_wchunk(1, eng=nc.scalar)  # idle queue at start
                        wc0_loaded = True
                    if nq == 0 and g + 1 >= 2 and g + 1 < NKG:
                        load_wchunk(g + 1)   # prefetch next weight chunk
                    for c in range(4):
                        kd = 4 * g + c
                        xt_t = xg[:, c, :]
                        nc.tensor.matmul(
                            q_ps, wq_sb[:, kd, 0:P], xt_t,
                            start=(kd == 0), stop=(kd == NDT - 1),
                        )
                        nc.tensor.matmul(
                            k_ps, wk_sb[:, kd, 0:P], xt_t,
                            start=(kd == 0), stop=(kd == NDT - 1),
                        )
                        for st in range(4):
                            nc.tensor.matmul(
                                v_ps[st], xg[:, c, st * P:(st + 1) * P],
                                wv_sb[:, kd, :],
                                start=(kd == 0), stop=(kd == NDT - 1),
                            )
                if nq == 0:
                    # pair-0 u table behind the weight chunks on gpsimd
                    # (phase C only needs it ~90us in)
                    u0 = load_u(0)
                # drain: V -> vaug blocks first (frees the 2 V psum slots the
                # next nq's V matmuls are waiting on), then q/k casts
                for st in range(4):
                    ktg = nq * 4 + st
                    vsrc = v_ps[st].rearrange("p (pr par d) -> p pr par d",
                                              par=2, d=D_KV)
                    nc.vector.tensor_copy(vaug[:, ktg, :, 0:D_KV],
                                          vsrc[:, :, 0, :])
                    nc.vector.tensor_copy(vaug[:, ktg, :, 129:193],
                                          vsrc[:, :, 1, :])
                nc.vector.tensor_copy(rev_chunk(qt_pq[0][NQC - 1 - nq][:, :]),
                                      q_ps)
                nc.vector.tensor_copy(kt_pq[0][nq][:, :], k_ps)

            # wo load issues from the idle scalar queue once phase B's
            # critical streams are done (needed only in phase D)
            nc.scalar.dma_start(wo_sb.rearrange("p a b -> p (a b)"), wo[:])

            # ---------- phase C: attention, proj of pair pr+1 interleaved ----
            def emit_sg(pr, qc, kt):
                """scores psum group for (pair, q-chunk, k-tile): the two
                heads run as concurrent 64-row-group tiles."""
                jg0 = qc * SC
                s01 = psum.tile([P, 2 * SC], DT.float32, tag="ps",
                                name=f"s{pr}_{qc}_{kt}")
                kc, ko = kt // 4, (kt % 4) * P
                nc.tensor.matmul(
                    s01[:, 0:SC], kt_pq[pr][kc][0:64, ko:ko + P],
                    qt_pq[pr][qc][0:64, :],
                    start=True, stop=True, tile_position=(0, 0),
                )
                nc.tensor.matmul(
                    s01[:, SC:2 * SC], kt_pq[pr][kc][64:128, ko:ko + P],
                    qt_pq[pr][qc][64:128, :],
                    start=True, stop=True, tile_position=(64, 0),
                )
                return s01

            # proj x^T group tiles, prefetched one group ahead.  The proj
            # processes q-ranges in order [0, 3, 1, 2] (indexed by the
            # attention qc): with the reversed-q store, this drains kt
            # chunk 0 and qt chunk 0 of the NEXT pair during its first two
            # q-chunks, so the next pair's first scores never wait on the
            # last drain at the pair boundary.
            PROJ_QC = [0, 3, 1, 2]

            def load_pgroup(proj, qc, g):
                pqc = PROJ_QC[qc]
                t = xtp.tile([P, 4, SC], DT.bfloat16, tag="xt",
                             name=f"xp{proj}_{qc}_{g}")
                nc.sync.dma_start(t[:], xt[:, pqc, 4 * g:4 * (g + 1), :])
                return t

            def attn_qc(pr, qc, u_t, proj, pending, s_pre, nxt_sg, pg0,
                        upf=None, px_pre=None):
                """attention for head pair pr, reversed-q chunk qc.
                proj: None or pr+1 (emit that pair's Q/K proj, 1 kd per kt).
                pg0: pre-loaded x^T group 0 for the proj (or None).
                Returns (normalize closure, pre-emitted next score group,
                pre-loaded group 0 for the NEXT (proj, qc), next-pair u)."""
                u_ret = None
                h0, h1 = 2 * pr, 2 * pr + 1
                jg0 = qc * SC
                cx01 = psum.tile([P, 2 * SC], DT.float32, tag="ps",
                                 name=f"cx{pr}_{qc}")
                if proj is not None:
                    pj_ps = psum.tile([P, 2 * SC], DT.float32, tag="ps",
                                      name=f"pjps{proj}_{qc}")
                    pjq, pjk = pj_ps[:, 0:SC], pj_ps[:, SC:2 * SC]
                    pgs = {0: pg0}

                def emit_proj(kt):
                    g, c = kt // 4, kt % 4
                    if c == 0 and g + 1 < NKG:
                        pgs[g + 1] = load_pgroup(proj, qc, g + 1)
                    kd = kt
                    xt_t = pgs[g][:, c, :]
                    nc.tensor.matmul(
                        pjq, wq_sb[:, kd, proj * P:(proj + 1) * P], xt_t,
                        start=(kd == 0), stop=(kd == NDT - 1),
                    )
                    nc.tensor.matmul(
                        pjk, wk_sb[:, kd, proj * P:(proj + 1) * P], xt_t,
                        start=(kd == 0), stop=(kd == NDT - 1),
                    )

                # 2-deep software pipeline: s(kt+2) is emitted before PV(kt)
                # so the in-order PE queue keeps a backlog (hides LDWEIGHTS
                # and cross-engine semaphore latency).  pending() emits the
                # PREVIOUS qc's deferred normalize chain (DVE+DMA only).
                sq = [s_pre if s_pre is not None else emit_sg(pr, qc, 0),
                      emit_sg(pr, qc, 1)]
                for kt in range(NKT):
                    if kt + 2 < NKT:
                        sq.append(emit_sg(pr, qc, kt + 2))
                    if proj is not None:
                        emit_proj(kt)
                    if kt == 2 and pending is not None:
                        pending[0]()
                    if kt == 8 and upf is not None:
                        u_ret = upf()
                    if kt == 12 and pending is not None:
                        pending[1]()
                    s01 = sq[kt]
                    if kt == 0 and px_pre is not None:
                        px = px_pre
                    else:
                        px = pexpp.tile([P, 2 * SC], DT.bfloat16, tag="pexp",
                                        name=f"px{pr}_{qc}_{kt}")
                        nc.scalar.activation(
                            out=px[:], in_=s01[:], func=AF.Exp,
                            bias=mask_sb[:, kt:kt + 1],
                            scale=1.0 / math.sqrt(D_KV),
                        )
                        j0 = kt * P + jg0
                        nc.vector.tensor_tensor(
                            px.rearrange("p (h j) -> p h j", h=2),
                            px.rearrange("p (h j) -> p h j", h=2),
                            u_t[:, :, j0:j0 + SC], OP.mult
                        )
                    nc.tensor.matmul(
                        cx01[0:65, 0:SC], vaug[:, kt, pr, 0:65], px[:, 0:SC],
                        start=(kt == 0), stop=(kt == NKT - 1),
                    )
                    nc.tensor.matmul(
                        cx01[:, SC:2 * SC], vaug[:, kt, pr, 65:VW],
                        px[:, SC:2 * SC],
                        start=(kt == 0), stop=(kt == NKT - 1),
                    )

                # proj drain (reversed q for qt)
                if proj is not None:
                    pqc = PROJ_QC[qc]
                    nc.scalar.copy(
                        rev_chunk(qt_pq[proj][NQC - 1 - pqc][:, :]), pjq)
                    nc.vector.tensor_copy(kt_pq[proj][pqc][:, :], pjk)

                # prefetch group 0 of the NEXT (proj, qc)'s x^T
                pg_next = None
                if qc + 1 < NQC and proj is not None:
                    pg_next = load_pgroup(proj, qc + 1, 0)
                elif qc == NQC - 1 and proj is not None and proj + 1 < NPAIR:
                    pg_next = load_pgroup(proj + 1, 0, 0)

                # pre-emit the NEXT qc's first score group AND its exp +
                # erel multiply, so the boundary Vector burst (proj drain +
                # cx evacuation) never delays the next qc's first PV
                s_next = None
                px_next = None
                if nxt_sg is not None:
                    s_next, npr, nqc, nu = nxt_sg()
                    px_next = pexpp.tile([P, 2 * SC], DT.bfloat16,
                                         tag="pexp", name=f"pxp{npr}_{nqc}")
                    nc.scalar.activation(
                        out=px_next[:], in_=s_next[:], func=AF.Exp,
                        bias=mask_sb[:, 0:1], scale=1.0 / math.sqrt(D_KV),
                    )
                    nc.vector.tensor_tensor(
                        px_next.rearrange("p (h j) -> p h j", h=2),
                        px_next.rearrange("p (h j) -> p h j", h=2),
                        nu[:, :, nqc * SC:nqc * SC + SC], OP.mult
                    )

                # ---- evacuate cx to SBUF (frees the PSUM slot), then the
                # rest of normalize+writeback is DEFERRED into the next qc
                # (DVE + DMA only; the PE never touches it) ----
                cxs = stage.tile([P, 2 * SC], DT.bfloat16, tag="cxs",
                                 name=f"cxs{pr}_{qc}", bufs=1)
                nc.vector.tensor_copy(cxs[:], cx01[:])

                bc_box = {}

                def norm_a():
                    # denominators: h0 on row 64 (cols 0:512), h1 on row 32
                    # (cols 512:1024).  Custom DVE ops need base-partition-0
                    # operands, so pack both rows into a base-0 tile first.
                    # Runs at kt==2 of the NEXT qc; the DRAM-bounce broadcast
                    # DMAs get ~8 k-tiles of latency slack before norm_b's
                    # tensor_tensors (at kt==8) consume bc_sb - the ~5us
                    # chain latency never blocks the Vector queue.
                    dnf = stage.tile([P, SC], DT.float32, tag="dnf",
                                     name=f"dnf{pr}_{qc}", bufs=1)
                    nc.vector.tensor_copy(dnf[64:65, :], cxs[64:65, 0:SC])
                    nc.vector.tensor_copy(dnf[32:33, :], cxs[32:33, SC:2 * SC])
                    rb = stage.tile([P, SC], DT.float32, tag="rb",
                                    name=f"rb{pr}_{qc}", bufs=1)
                    nc.vector.reciprocal_approx_fast(out=rb[:], in_=dnf[:])
                    rbh = stage.tile([P, SC], DT.bfloat16, tag="rbh",
                                     name=f"rbh{pr}_{qc}", bufs=1)
                    nc.vector.tensor_copy(rbh[64:65, :], rb[64:65, :])
                    nc.vector.tensor_copy(rbh[32:33, :], rb[32:33, :])
                    # broadcast across partitions: bounce the reciprocal
                    # rows through DRAM, then stride-0 reads replicate them.
                    # norm_b consumes bc_sb only at kt==12, so the DMA chain
                    # has ~13us of slack before it can block Vector.
                    bnc = dramp.tile([2, SC], DT.bfloat16, tag="bnc",
                                     name=f"bnc{pr}_{qc}")
                    nc.sync.dma_start(bnc[0:1, :], rbh[64:65, :])
                    nc.sync.dma_start(bnc[1:2, :], rbh[32:33, :])
                    bc_sb = stage.tile([P, SC], DT.bfloat16, tag="bc",
                                       name=f"bcs{pr}_{qc}", bufs=2)
                    src0 = bass.AP(tensor=bnc.tensor, offset=bnc.offset,
                                   ap=[[0, 64], [1, SC]])
                    src1 = bass.AP(tensor=bnc.tensor, offset=bnc.offset + SC,
                                   ap=[[0, 64], [1, SC]])
                    nc.sync.dma_start(bc_sb[0:64, :], src0)
                    nc.sync.dma_start(bc_sb[64:128, :], src1)
                    # (whole bounce rides sync/HWDGE: SWDGE completions
                    # took ~5-8us under load behind the 2MB u-table loads,
                    # starving norm_b and head-of-line blocking Vector)
                    bc_box["bc"] = bc_sb

                def norm_b():
                    bc_sb = bc_box["bc"]
                    ct = ctxt_pq[pr][NQC - 1 - qc]
                    nc.vector.tensor_tensor(
                        rev_chunk(ct[0:64, :]),
                        cxs[0:64, 0:SC], bc_sb[0:64, :], OP.mult)
                    nc.vector.tensor_tensor(
                        rev_chunk(ct[64:128, :]),
                        cxs[64:128, SC:2 * SC], bc_sb[64:128, :], OP.mult)
                return (norm_a, norm_b), s_next, px_next, pg_next, u_ret

            u_t = u0  # pair-0 table already loaded during phase B
            pending = None
            s_pre = None
            px_pre = None
            pg0 = load_pgroup(1, 0, 0)
            seq = [(pr, qc) for pr in range(NPAIR) for qc in range(NQC)]
            next_u = None
            for idx, (pr, qc) in enumerate(seq):
                nxt = pr + 1 if pr + 1 < NPAIR else None
                if nxt is not None and qc == 0:
                    upf = (lambda nxt=nxt: load_u(nxt))
                else:
                    upf = None
                if idx + 1 < len(seq):
                    npr, nqc = seq[idx + 1]
                    # the u table the NEXT (pr,qc) will multiply with: at a
                    # pair boundary it is the freshly loaded next_u
                    nxt_sg = (lambda npr=npr, nqc=nqc:
                              (emit_sg(npr, nqc, 0), npr, nqc,
                               u_t if npr == pr else next_u))
                else:
                    nxt_sg = None
                pending, s_pre, px_pre, pg0, u_ret = attn_qc(
                    pr, qc, u_t, nxt, pending, s_pre, nxt_sg, pg0, upf,
                    px_pre)
                if u_ret is not None:
                    next_u = u_ret
                if qc == NQC - 1 and nxt is not None:
                    u_t = next_u
                    next_u = None
            pending[0]()
            pending[1]()

            # ---------- phase D: output projection (descending st: the
            # low-st tiles depend on the last deferred normalize) ----------
            for st in range(NKT - 1, -1, -1):
                oa = psum.tile([P, 2 * SC], DT.float32, tag="ps",
                               name=f"oa{st}")
                ob = psum.tile([P, 2 * SC], DT.float32, tag="ps",
                               name=f"ob{st}")
                o_ps = [oa[:, 0:SC], oa[:, SC:2 * SC],
                        ob[:, 0:SC], ob[:, SC:2 * SC]]
                for m in range(NMT):
                    for nd in range(NQC):
                        nc.tensor.matmul(
                            o_ps[nd],
                            ctxt_pq[m][st // 4][:, (st % 4) * P:
                                                (st % 4) * P + P],
                            wo_sb[:, m, nd * SC:(nd + 1) * SC],
                            start=(m == 0), stop=(m == NMT - 1),
                        )
                for half in range(2):
                    # separate staging tiles per engine: a shared tile
                    # serialized the vector copy behind the scalar copy
                    o_a = outp.tile([P, SC], DT.float32, tag="outa",
                                    name=f"ota{st}_{half}")
                    o_b = outp.tile([P, SC], DT.float32, tag="outb",
                                    name=f"otb{st}_{half}")
                    nc.scalar.copy(o_a[:], o_ps[2 * half])
                    nc.vector.tensor_copy(o_b[:], o_ps[2 * half + 1])
                    eng = nc.sync if half == 0 else nc.gpsimd
                    c0 = half * 2 * SC
                    eng.dma_start(
                        out[st * P:(st + 1) * P, c0:c0 + SC], o_a[:])
                    eng.dma_start(
                        out[st * P:(st + 1) * P, c0 + SC:c0 + 2 * SC], o_b[:])

    nc.finalize()
    return nc


_NC_CACHE = None


def _get_nc():
    global _NC_CACHE
    if _NC_CACHE is None:
        _NC_CACHE = _build()
    return _NC_CACHE


def _in_maps(hidden_states, attention_mask, Wq, Wk, Wv, Wo, rel_emb):
    import ml_dtypes
    bf16 = ml_dtypes.bfloat16
    maps = []
    for c in range(NCORES):
        b, g = c // 4, c % 4
        hlo, hhi = g * HL, (g + 1) * HL
        erel = _bias_table(
            np.ascontiguousarray(rel_emb[:, hlo:hhi], dtype=np.float32))
        def shuf(w):  # [NDT*P, C] -> [P, NDT*C] partition-contiguous
            cc = w.shape[1]
            return np.ascontiguousarray(
                w.reshape(-1, P, cc).transpose(1, 0, 2).reshape(P, -1))
        # xt[p, qc, kd, j] = x[qc*512+j, kd*128+p]
        xtt = np.ascontiguousarray(
            hidden_states[b].reshape(NQC, SC, NDT, P).transpose(3, 0, 2, 1)
        ).astype(bf16)
        maps.append({
            "xt": xtt,
            "wq": shuf(Wq[:, hlo * D_KV:hhi * D_KV]).astype(bf16),
            "wk": shuf(Wk[:, hlo * D_KV:hhi * D_KV]).astype(bf16),
            "wv": shuf(Wv[:, hlo * D_KV:hhi * D_KV]).astype(bf16),
            "wo": shuf(Wo[hlo * D_KV:hhi * D_KV, :]).astype(bf16),
            "mask": np.ascontiguousarray(attention_mask[b, 0, 0, :]).astype(np.float32),
            "erel": erel,
        })
    return maps


def kernel(hidden_states, attention_mask, Wq, Wk, Wv, Wo, rel_emb, _trace=False,
           _trace_kwargs=None):
    hidden_states = np.asarray(hidden_states, dtype=np.float32)
    attention_mask = np.asarray(attention_mask, dtype=np.float32)
    Wq = np.asarray(Wq, dtype=np.float32)
    Wk = np.asarray(Wk, dtype=np.float32)
    Wv = np.asarray(Wv, dtype=np.float32)
    Wo = np.asarray(Wo, dtype=np.float32)
    rel_emb = np.asarray(rel_emb, dtype=np.float32)

    nc = _get_nc()
    maps = _in_maps(hidden_states, attention_mask, Wq, Wk, Wv, Wo, rel_emb)
    kw = dict(_trace_kwargs or {})
    res = run_bass_kernel_spmd(nc, maps, core_ids=list(range(NCORES)),
                               trace=_trace, **kw)
    kernel.last_results = res
    outp = np.empty((B, S, D), dtype=np.float32)
    for b in range(B):
        acc = np.asarray(res.results[4 * b]["out"], dtype=np.float32).copy()
        for g in range(1, 4):
            acc += np.asarray(res.results[4 * b + g]["out"], dtype=np.float32)
        outp[b] = acc
    return outp


# revision 29
# speedup vs baseline: 1.0582x; 1.0087x over previous
"""T5-style encoder self-attention (dense_transformer) on 8 Trainium2 NeuronCores.

Problem (full shapes): hidden [2,2048,2048], Wq/Wk/Wv/Wo [2048,2048],
rel_emb [32,32] (bidirectional T5 relative-position bias), mask [2,1,1,2048].

Sharding: data-parallel over batch (2) x tensor-parallel over heads (4 groups
of 8 heads) = 8 cores, Megatron-style. Each core computes a partial output
[2048,2048] for its batch (its 8 heads through its Wo row-slice); the host
sums 4 partials per batch.

Per-core kernel design (bf16 operands, fp32 PSUM accumulation), ~518us HW:
  - Relative-position bias applied MULTIPLICATIVELY for EVERY tile:
    px = exp(s/8 + mask) * erel, erel = exp(bias) host-computed as a
    [8 heads, 4096 diagonals] bf16 table read through a Toeplitz shear AP
    (partition stride 1, free stride 1; 2 DMAs per pair, ~1MB each).  This
    removed the old near-tile PSUM identity-injection (~38us of PE) and its
    16us identity-matrix DMA on the startup critical path.
  - Phase B: single x^T pass computes pair-0 Q^T/K^T + V for ALL heads.
    Q^T stored s-REVERSED so the bias shear has positive strides.  x^T is
    host-tiled [128, qc, kd, 512] so each [128,4,512] group load is one DMA
    with 4KB partition lines; weight chunk 0 rides the sync queue right
    behind the first x group (first matmul ~16us in), chunks 1-3 stream on
    gpsimd in kd order; wo loads at phase-B end from the scalar queue.
  - Phase C attention per (head-pair, q-chunk), k-tile loop 2 deep:
      * per-head QK matmuls run as concurrent 64-row-group tiles;
      * ACT does exp in one [128,1024] shot per kt; DVE multiplies by the
        erel shear slice; PV accumulates ctx + ones-column denominators;
      * pair pr+1's Q/K projections interleave 1 kd per kt, q-chunks
        processed in order [0,3,1,2] so (with the reversed-q store) the
        NEXT pair's first-chunk qt/kt are drained two q-chunks before the
        pair boundary - the boundary never waits a drain;
      * qt/kt/ctxt live as per-(pair, q-chunk) [128,512] tiles: the dep
        tracker is not interval-precise inside big tensors and was
        serializing score reads behind unrelated drain writes;
      * the next qc's first score group AND its exp + erel multiply are
        pre-emitted in the current tail so the boundary Vector burst
        (drains + cx evacuation) never delays the next qc's first PV.
  - Normalize is deferred and PE-free, split in two stages: at kt==2 of the
    next qc, DVE packs the two denominator rows, reciprocal_approx_fast,
    bf16-casts, then a DRAM bounce broadcasts them across partitions
    (2 writes + 2 stride-0 reads, ALL on the sync/HWDGE queue - SWDGE
    completions took 5-8us under load and head-of-line blocked Vector);
    at kt==12 two fused tensor_tensors do normalize + un-reverse + bf16
    writeback into ctxt.  bc/bnc staging is double-buffered so chains of
    adjacent qcs never serialize.
  - Phase D output projection: descending s-tiles (low tiles depend on the
    last normalize), m inside nd for PSUM bank spread; evacuation uses
    SEPARATE per-engine staging tiles (a shared tile serialized DVE behind
    ACT); out DMAs alternate sync/gpsimd queues.
  - DMA queue discipline: sync = x-tile groups + proj prefetch + normalize
    bounce; gpsimd = weight chunks, u tables, phase-D half; scalar = mask,
    ACT-table warmup, wo.  Few LARGE DMAs beat many small ones: each
    dma_start costs ~600ns of queue issue time, and the scheduler bunches
    split loads into queue-hogging bursts regardless of emission position.
"""

import math
import sys

for _p in ("/opt/trn_rl_repo",):
    if _p not in sys.path:
        sys.path.insert(0, _p)

import numpy as np

import concourse.bass as bass
import concourse.mybir as mybir
import concourse.tile as tile
from concourse import bacc
from concourse.bass_utils import run_bass_kernel_spmd

DT = mybir.dt
AF = mybir.ActivationFunctionType
OP = mybir.AluOpType

# ---- problem constants (hardcoded per contract) ----
B, S, D = 2, 2048, 2048
N_HEADS, D_KV = 32, 64
NUM_BUCKETS, MAX_DISTANCE = 32, 128
NCORES = 8
HL = 8            # heads per core
P = 128
SC = 512          # free-dim chunk
NKT = S // P      # 16 k-tiles
NQC = S // SC     # 4 q-chunks
NDT = D // P      # 16 D-tiles
NMT = (HL * D_KV) // P   # 4 hd m-tiles per core
NPAIR = HL // 2   # 4 head pairs per core
NDIAG = 4096
W_U = 3968        # erel shear tile width (covers all diagonals any tile hits)
VW = 193          # vaug per-(kt,pair) width: even block 65 + odd block 128
NKG = NDT // 4    # 4 kd-groups of 4 chunks per q-chunk (x^T group loads)


def _rel_bucket_host(d):
    """Exact numpy replica of reference._relative_position_bucket."""
    num_buckets = NUM_BUCKETS // 2          # 16
    max_exact = num_buckets // 2            # 8
    rel = np.asarray(d, dtype=np.int64)
    buckets = (rel > 0).astype(np.int32) * num_buckets
    arel = np.abs(rel)
    is_small = arel < max_exact
    rp_safe = np.maximum(arel, 1).astype(np.float32)
    log_ratio = np.log(rp_safe / np.float32(max_exact)).astype(np.float32)
    scale = np.float32(math.log(MAX_DISTANCE / max_exact))
    rp_large = max_exact + (log_ratio / scale * np.float32(num_buckets - max_exact)).astype(np.int32)
    rp_large = np.minimum(rp_large, num_buckets - 1)
    buckets = buckets + np.where(is_small, arel.astype(np.int32), rp_large)
    return buckets.astype(np.int32)


def _bias_table(rel_emb_slice):
    """rel_emb_slice: [NUM_BUCKETS, HL] fp32 -> erel [HL, NDIAG] bf16,
    erel[h, i] = exp(bias(d = i - 2047)); erel[:, 4095] is never read."""
    import ml_dtypes
    i = np.arange(NDIAG - 1)
    b = _rel_bucket_host(i - (S - 1))                  # [4095]
    vals = rel_emb_slice[b, :]                         # [4095, HL] fp32
    erel = np.zeros((HL, NDIAG), dtype=np.float32)
    erel[:, : NDIAG - 1] = np.exp(vals.T)
    return erel.astype(ml_dtypes.bfloat16)


def _build():
    nc = bacc.Bacc(None, name="attn_tp")

    # x^T host-tiled: xt[p, qc, kd, j] = x[qc*512+j, kd*128+p], so a
    # [128, 4, 512] kd-group load is one DMA with 4KB per-partition lines
    xt = nc.declare_dram_parameter("xt", [P, NQC, NDT, SC], DT.bfloat16,
                                   isOutput=False)
    # weights arrive HOST-SHUFFLED to [p][kt][h] so per-partition lines are
    # contiguous multi-KB runs (DMA packet rate is the limiter at 1KB lines)
    wq = nc.declare_dram_parameter("wq", [P, NDT * HL * D_KV], DT.bfloat16, isOutput=False)
    wk = nc.declare_dram_parameter("wk", [P, NDT * HL * D_KV], DT.bfloat16, isOutput=False)
    wv = nc.declare_dram_parameter("wv", [P, NDT * HL * D_KV], DT.bfloat16, isOutput=False)
    wo = nc.declare_dram_parameter("wo", [P, NMT * D], DT.bfloat16, isOutput=False)
    mask = nc.declare_dram_parameter("mask", [S], DT.float32, isOutput=False)
    erel = nc.declare_dram_parameter("erel", [HL, NDIAG], DT.bfloat16, isOutput=False)
    out = nc.declare_dram_parameter("out", [S, D], DT.float32, isOutput=True)

    with tile.TileContext(nc) as tc:
        with (
            tc.tile_pool(name="res", bufs=1) as res,          # persistent tensors
            tc.tile_pool(name="xtp", bufs=3) as xtp,          # x^T groups (sync q)
            tc.tile_pool(name="upool", bufs=2) as upool,      # exp-bias shear tiles
            tc.tile_pool(name="pexp", bufs=3) as pexpp,       # probs tiles
            tc.tile_pool(name="stage", bufs=2) as stage,      # normalize staging
            tc.tile_pool(name="outp", bufs=3) as outp,        # out staging
            tc.tile_pool(name="psum", bufs=4, space="PSUM") as psum,  # [128,1024] slots
            tc.tile_pool(name="dram", bufs=2, space="DRAM") as dramp,
        ):
            # ---------- constants / resident tensors ----------
            mask_sb = res.tile([P, NKT], DT.float32, tag="mask")
            # mask + ACT exp-table warm-up ride the (otherwise idle) scalar
            # queue so the sync/gpsimd queues start with the critical loads
            nc.scalar.dma_start(mask_sb[:], mask.ap().rearrange("(kt p) -> p kt", p=P))

            wq_sb = res.tile([P, NDT, HL * D_KV], DT.bfloat16, tag="wq")
            wk_sb = res.tile([P, NDT, HL * D_KV], DT.bfloat16, tag="wk")
            wv_sb = res.tile([P, NDT, HL * D_KV], DT.bfloat16, tag="wv")
            wo_sb = res.tile([P, NMT, D], DT.bfloat16, tag="wo")

            # persistent activations.  qt/kt/ctxt are split per (pair,
            # q-chunk) so the tile dep tracker never serializes score /
            # phase-D reads behind a LATER chunk's drain writes (the
            # tracker is not interval-precise across a big tensor).
            qt_pq = [[res.tile([P, SC], DT.bfloat16, tag=f"qt{m}_{c}",
                               name=f"qt{m}_{c}") for c in range(NQC)]
                     for m in range(NMT)]                      # q REVERSED
            kt_pq = [[res.tile([P, SC], DT.bfloat16, tag=f"kt{m}_{c}",
                               name=f"kt{m}_{c}") for c in range(NQC)]
                     for m in range(NMT)]
            vaug = res.tile([P, NKT, NPAIR, VW], DT.bfloat16, tag="vaug")
            ctxt_pq = [[res.tile([P, SC], DT.bfloat16, tag=f"ct{m}_{c}",
                                 name=f"ct{m}_{c}") for c in range(NQC)]
                       for m in range(NMT)]
            # only the two ones-columns are ever read outside the V blocks
            # (psum rows other than the denominator rows are never consumed)
            nc.vector.memset(vaug[:, :, :, 64:65], 1.0)
            nc.vector.memset(vaug[:, :, :, 97:98], 1.0)

            # ACT exp table warm-up (hide the ~2.7us table load under phase B)
            warm = res.tile([1, 2], DT.float32, tag="warm")
            nc.scalar.activation(out=warm[0:1, 0:1], in_=mask_sb[0:1, 0:1], func=AF.Exp)

            def rev_chunk(t):
                """reversed-q view over one [rows, SC] chunk tile: writing
                reversed data lands in natural order.  A logical q-chunk qc
                written reversed covers NATURAL chunk NQC-1-qc."""
                return bass.AP(
                    tensor=t.tensor,
                    offset=t.offset + (SC - 1),
                    ap=[list(t.ap[0]), [-1, SC]],
                )

            def load_u(pr, eng=None):
                """erel shear tile [P, 2, W_U] for pair pr: u[p, i, w] =
                erel[2*pr+i, p + w].  One DMA per head: fewer, larger DMAs
                keep the issuing queue free (the scheduler bunches split
                loads into queue-hogging bursts anyway)."""
                u = upool.tile([P, 2, W_U], DT.bfloat16, tag="u",
                               name=f"u{pr}", bufs=2)
                ap0 = erel.ap()
                for i, hh in enumerate((2 * pr, 2 * pr + 1)):
                    shear = bass.AP(
                        tensor=ap0.tensor,
                        offset=ap0.offset + hh * NDIAG,
                        ap=[[1, P], [1, W_U]],
                    )
                    (eng or nc.gpsimd).dma_start(u[:, i, :], shear)
                return u

            def load_wchunk(g, eng=None):
                """one 4-kd chunk of weights; chunk 0 rides sync (HWDGE,
                ~0.6us first byte) right behind the first x group so the
                first matmuls fire ~10us in; later chunks stream on gpsimd
                in kd order."""
                eng = eng or nc.gpsimd
                cw = HL * D_KV
                c0, c1 = g * 4 * cw, (g + 1) * 4 * cw
                eng.dma_start(wq_sb[:, g * 4:(g + 1) * 4, :], wq[:, c0:c1])
                eng.dma_start(wk_sb[:, g * 4:(g + 1) * 4, :], wk[:, c0:c1])
                eng.dma_start(wv_sb[:, g * 4:(g + 1) * 4, :], wv[:, c0:c1])

            def load_xgroup(nq, g):
                """one [128, 4, 512] x^T kd-group (4KB partition lines)."""
                t = xtp.tile([P, 4, SC], DT.bfloat16, tag="xt",
                             name=f"xg{nq}_{g}")
                nc.sync.dma_start(t[:], xt[:, nq, 4 * g:4 * (g + 1), :])
                return t

            # ---------- phase B: pair-0 Q/K + V (all heads), single x^T pass ----
            wc0_loaded = False
            for nq in range(NQC):
                qk_ps = psum.tile([P, 2 * SC], DT.float32, tag="ps",
                                  name=f"qkps0_{nq}")
                q_ps, k_ps = qk_ps[:, 0:SC], qk_ps[:, SC:2 * SC]
                v01 = psum.tile([P, 2 * SC], DT.float32, tag="ps", name=f"v01_{nq}")
                v23 = psum.tile([P, 2 * SC], DT.float32, tag="ps", name=f"v23_{nq}")
                v_ps = [v01[:, 0:SC], v01[:, SC:2 * SC],
                        v23[:, 0:SC], v23[:, SC:2 * SC]]
                for g in range(NKG):
                    xg = load_xgroup(nq, g)
                    if nq == 0 and not wc0_loaded:
                        load_wchunk(0, eng=nc.sync)
                        wc0_loaded = True
                    if nq == 0 and g + 1 < NKG:
                        load_wchunk(g + 1)   # prefetch next weight chunk
                    for c in range(4):
                        kd = 4 * g + c
                        xt_t = xg[:, c, :]
                        nc.tensor.matmul(
                            q_ps, wq_sb[:, kd, 0:P], xt_t,
                            start=(kd == 0), stop=(kd == NDT - 1),
                        )
                        nc.tensor.matmul(
                            k_ps, wk_sb[:, kd, 0:P], xt_t,
                            start=(kd == 0), stop=(kd == NDT - 1),
                        )
                        for st in range(4):
                            nc.tensor.matmul(
                                v_ps[st], xg[:, c, st * P:(st + 1) * P],
                                wv_sb[:, kd, :],
                                start=(kd == 0), stop=(kd == NDT - 1),
                            )
                if nq == 0:
                    # pair-0 u table behind the weight chunks on gpsimd
                    # (phase C only needs it ~90us in)
                    u0 = load_u(0)
                # drain: V -> vaug blocks first (frees the 2 V psum slots the
                # next nq's V matmuls are waiting on), then q/k casts
                for st in range(4):
                    ktg = nq * 4 + st
                    vsrc = v_ps[st].rearrange("p (pr par d) -> p pr par d",
                                              par=2, d=D_KV)
                    nc.vector.tensor_copy(vaug[:, ktg, :, 0:D_KV],
                                          vsrc[:, :, 0, :])
                    nc.vector.tensor_copy(vaug[:, ktg, :, 129:193],
                                          vsrc[:, :, 1, :])
                nc.vector.tensor_copy(rev_chunk(qt_pq[0][NQC - 1 - nq][:, :]),
                                      q_ps)
                nc.vector.tensor_copy(kt_pq[0][nq][:, :], k_ps)

            # wo load issues from the idle scalar queue once phase B's
            # critical streams are done (needed only in phase D)
            nc.scalar.dma_start(wo_sb.rearrange("p a b -> p (a b)"), wo[:])

            # ---------- phase C: attention, proj of pair pr+1 interleaved ----
            def emit_sg(pr, qc, kt):
                """scores psum group for (pair, q-chunk, k-tile): the two
                heads run as concurrent 64-row-group tiles."""
                jg0 = qc * SC
                s01 = psum.tile([P, 2 * SC], DT.float32, tag="ps",
                                name=f"s{pr}_{qc}_{kt}")
                kc, ko = kt // 4, (kt % 4) * P
                nc.tensor.matmul(
                    s01[:, 0:SC], kt_pq[pr][kc][0:64, ko:ko + P],
                    qt_pq[pr][qc][0:64, :],
                    start=True, stop=True, tile_position=(0, 0),
                )
                nc.tensor.matmul(
                    s01[:, SC:2 * SC], kt_pq[pr][kc][64:128, ko:ko + P],
                    qt_pq[pr][qc][64:128, :],
                    start=True, stop=True, tile_position=(64, 0),
                )
                return s01

            # proj x^T group tiles, prefetched one group ahead.  The proj
            # processes q-ranges in order [0, 3, 1, 2] (indexed by the
            # attention qc): with the reversed-q store, this drains kt
            # chunk 0 and qt chunk 0 of the NEXT pair during its first two
            # q-chunks, so the next pair's first scores never wait on the
            # last drain at the pair boundary.
            PROJ_QC = [0, 3, 1, 2]

            def load_pgroup(proj, qc, g):
                pqc = PROJ_QC[qc]
                t = xtp.tile([P, 4, SC], DT.bfloat16, tag="xt",
                             name=f"xp{proj}_{qc}_{g}")
                nc.sync.dma_start(t[:], xt[:, pqc, 4 * g:4 * (g + 1), :])
                return t

            def attn_qc(pr, qc, u_t, proj, pending, s_pre, nxt_sg, pg0,
                        upf=None, px_pre=None):
                """attention for head pair pr, reversed-q chunk qc.
                proj: None or pr+1 (emit that pair's Q/K proj, 1 kd per kt).
                pg0: pre-loaded x^T group 0 for the proj (or None).
                Returns (normalize closure, pre-emitted next score group,
                pre-loaded group 0 for the NEXT (proj, qc), next-pair u)."""
                u_ret = None
                h0, h1 = 2 * pr, 2 * pr + 1
                jg0 = qc * SC
                cx01 = psum.tile([P, 2 * SC], DT.float32, tag="ps",
                                 name=f"cx{pr}_{qc}")
                if proj is not None:
                    pj_ps = psum.tile([P, 2 * SC], DT.float32, tag="ps",
                                      name=f"pjps{proj}_{qc}")
                    pjq, pjk = pj_ps[:, 0:SC], pj_ps[:, SC:2 * SC]
                    pgs = {0: pg0}

                def emit_proj(kt):
                    g, c = kt // 4, kt % 4
                    if c == 0 and g + 1 < NKG:
                        pgs[g + 1] = load_pgroup(proj, qc, g + 1)
                    kd = kt
                    xt_t = pgs[g][:, c, :]
                    nc.tensor.matmul(
                        pjq, wq_sb[:, kd, proj * P:(proj + 1) * P], xt_t,
                        start=(kd == 0), stop=(kd == NDT - 1),
                    )
                    nc.tensor.matmul(
                        pjk, wk_sb[:, kd, proj * P:(proj + 1) * P], xt_t,
                        start=(kd == 0), stop=(kd == NDT - 1),
                    )

                # 2-deep software pipeline: s(kt+2) is emitted before PV(kt)
                # so the in-order PE queue keeps a backlog (hides LDWEIGHTS
                # and cross-engine semaphore latency).  pending() emits the
                # PREVIOUS qc's deferred normalize chain (DVE+DMA only).
                sq = [s_pre if s_pre is not None else emit_sg(pr, qc, 0),
                      emit_sg(pr, qc, 1)]
                for kt in range(NKT):
                    if kt + 2 < NKT:
                        sq.append(emit_sg(pr, qc, kt + 2))
                    if proj is not None:
                        emit_proj(kt)
                    if kt == 2 and pending is not None:
                        pending[0]()
                    if kt == 8 and upf is not None:
                        u_ret = upf()
                    if kt == 12 and pending is not None:
                        pending[1]()
                    s01 = sq[kt]
                    if kt == 0 and px_pre is not None:
                        px = px_pre
                    else:
                        px = pexpp.tile([P, 2 * SC], DT.bfloat16, tag="pexp",
                                        name=f"px{pr}_{qc}_{kt}")
                        nc.scalar.activation(
                            out=px[:], in_=s01[:], func=AF.Exp,
                            bias=mask_sb[:, kt:kt + 1],
                            scale=1.0 / math.sqrt(D_KV),
                        )
                        j0 = kt * P + jg0
                        nc.vector.tensor_tensor(
                            px.rearrange("p (h j) -> p h j", h=2),
                            px.rearrange("p (h j) -> p h j", h=2),
                            u_t[:, :, j0:j0 + SC], OP.mult
                        )
                    nc.tensor.matmul(
                        cx01[0:65, 0:SC], vaug[:, kt, pr, 0:65], px[:, 0:SC],
                        start=(kt == 0), stop=(kt == NKT - 1),
                    )
                    nc.tensor.matmul(
                        cx01[:, SC:2 * SC], vaug[:, kt, pr, 65:VW],
                        px[:, SC:2 * SC],
                        start=(kt == 0), stop=(kt == NKT - 1),
                    )

                # proj drain (reversed q for qt)
                if proj is not None:
                    pqc = PROJ_QC[qc]
                    nc.scalar.copy(
                        rev_chunk(qt_pq[proj][NQC - 1 - pqc][:, :]), pjq)
                    nc.vector.tensor_copy(kt_pq[proj][pqc][:, :], pjk)

                # prefetch group 0 of the NEXT (proj, qc)'s x^T
                pg_next = None
                if qc + 1 < NQC and proj is not None:
                    pg_next = load_pgroup(proj, qc + 1, 0)
                elif qc == NQC - 1 and proj is not None and proj + 1 < NPAIR:
                    pg_next = load_pgroup(proj + 1, 0, 0)

                # pre-emit the NEXT qc's first score group AND its exp +
                # erel multiply, so the boundary Vector burst (proj drain +
                # cx evacuation) never delays the next qc's first PV
                s_next = None
                px_next = None
                if nxt_sg is not None:
                    s_next, npr, nqc, nu = nxt_sg()
                    px_next = pexpp.tile([P, 2 * SC], DT.bfloat16,
                                         tag="pexp", name=f"pxp{npr}_{nqc}")
                    nc.scalar.activation(
                        out=px_next[:], in_=s_next[:], func=AF.Exp,
                        bias=mask_sb[:, 0:1], scale=1.0 / math.sqrt(D_KV),
                    )
                    nc.vector.tensor_tensor(
                        px_next.rearrange("p (h j) -> p h j", h=2),
                        px_next.rearrange("p (h j) -> p h j", h=2),
                        nu[:, :, nqc * SC:nqc * SC + SC], OP.mult
                    )

                # ---- evacuate cx to SBUF (frees the PSUM slot), then the
                # rest of normalize+writeback is DEFERRED into the next qc
                # (DVE + DMA only; the PE never touches it) ----
                cxs = stage.tile([P, 2 * SC], DT.bfloat16, tag="cxs",
                                 name=f"cxs{pr}_{qc}", bufs=1)
                nc.vector.tensor_copy(cxs[:], cx01[:])

                bc_box = {}

                def norm_a():
                    # denominators: h0 on row 64 (cols 0:512), h1 on row 32
                    # (cols 512:1024).  Custom DVE ops need base-partition-0
                    # operands, so pack both rows into a base-0 tile first.
                    # Runs at kt==2 of the NEXT qc; the DRAM-bounce broadcast
                    # DMAs get ~8 k-tiles of latency slack before norm_b's
                    # tensor_tensors (at kt==8) consume bc_sb - the ~5us
                    # chain latency never blocks the Vector queue.
                    dnf = stage.tile([P, SC], DT.float32, tag="dnf",
                                     name=f"dnf{pr}_{qc}", bufs=1)
                    nc.vector.tensor_copy(dnf[64:65, :], cxs[64:65, 0:SC])
                    nc.vector.tensor_copy(dnf[32:33, :], cxs[32:33, SC:2 * SC])
                    rb = stage.tile([P, SC], DT.float32, tag="rb",
                                    name=f"rb{pr}_{qc}", bufs=1)
                    nc.vector.reciprocal_approx_fast(out=rb[:], in_=dnf[:])
                    rbh = stage.tile([P, SC], DT.bfloat16, tag="rbh",
                                     name=f"rbh{pr}_{qc}", bufs=1)
                    nc.vector.tensor_copy(rbh[64:65, :], rb[64:65, :])
                    nc.vector.tensor_copy(rbh[32:33, :], rb[32:33, :])
                    # broadcast across partitions: bounce the reciprocal
                    # rows through DRAM, then stride-0 reads replicate them.
                    # norm_b consumes bc_sb only at kt==12, so the DMA chain
                    # has ~13us of slack before it can block Vector.
                    bnc = dramp.tile([2, SC], DT.bfloat16, tag="bnc",
                                     name=f"bnc{pr}_{qc}")
                    nc.sync.dma_start(bnc[0:1, :], rbh[64:65, :])
                    nc.sync.dma_start(bnc[1:2, :], rbh[32:33, :])
                    bc_sb = stage.tile([P, SC], DT.bfloat16, tag="bc",
                                       name=f"bcs{pr}_{qc}", bufs=2)
                    src0 = bass.AP(tensor=bnc.tensor, offset=bnc.offset,
                                   ap=[[0, 64], [1, SC]])
                    src1 = bass.AP(tensor=bnc.tensor, offset=bnc.offset + SC,
                                   ap=[[0, 64], [1, SC]])
                    nc.sync.dma_start(bc_sb[0:64, :], src0)
                    nc.sync.dma_start(bc_sb[64:128, :], src1)
                    # (whole bounce rides sync/HWDGE: SWDGE completions
                    # took ~5-8us under load behind the 2MB u-table loads,
                    # starving norm_b and head-of-line blocking Vector)
                    bc_box["bc"] = bc_sb

                def norm_b():
                    bc_sb = bc_box["bc"]
                    ct = ctxt_pq[pr][NQC - 1 - qc]
                    nc.vector.tensor_tensor(
                        rev_chunk(ct[0:64, :]),
                        cxs[0:64, 0:SC], bc_sb[0:64, :], OP.mult)
                    nc.vector.tensor_tensor(
                        rev_chunk(ct[64:128, :]),
                        cxs[64:128, SC:2 * SC], bc_sb[64:128, :], OP.mult)
                return (norm_a, norm_b), s_next, px_next, pg_next, u_ret

            u_t = u0  # pair-0 table already loaded during phase B
            pending = None
            s_pre = None
            px_pre = None
            pg0 = load_pgroup(1, 0, 0)
            seq = [(pr, qc) for pr in range(NPAIR) for qc in range(NQC)]
            next_u = None
            for idx, (pr, qc) in enumerate(seq):
                nxt = pr + 1 if pr + 1 < NPAIR else None
                if nxt is not None and qc == 0:
                    upf = (lambda nxt=nxt: load_u(nxt))
                else:
                    upf = None
                if idx + 1 < len(seq):
                    npr, nqc = seq[idx + 1]
                    # the u table the NEXT (pr,qc) will multiply with: at a
                    # pair boundary it is the freshly loaded next_u
                    nxt_sg = (lambda npr=npr, nqc=nqc:
                              (emit_sg(npr, nqc, 0), npr, nqc,
                               u_t if npr == pr else next_u))
                else:
                    nxt_sg = None
                pending, s_pre, px_pre, pg0, u_ret = attn_qc(
                    pr, qc, u_t, nxt, pending, s_pre, nxt_sg, pg0, upf,
                    px_pre)
                if u_ret is not None:
                    next_u = u_ret
                if qc == NQC - 1 and nxt is not None:
                    u_t = next_u
                    next_u = None
            pending[0]()
            pending[1]()

            # ---------- phase D: output projection (descending st: the
            # low-st tiles depend on the last deferred normalize) ----------
            for st in range(NKT - 1, -1, -1):
                oa = psum.tile([P, 2 * SC], DT.float32, tag="ps",
                               name=f"oa{st}")
                ob = psum.tile([P, 2 * SC], DT.float32, tag="ps",
                               name=f"ob{st}")
                o_ps = [oa[:, 0:SC], oa[:, SC:2 * SC],
                        ob[:, 0:SC], ob[:, SC:2 * SC]]
                for m in range(NMT):
                    for nd in range(NQC):
                        nc.tensor.matmul(
                            o_ps[nd],
                            ctxt_pq[m][st // 4][:, (st % 4) * P:
                                                (st % 4) * P + P],
                            wo_sb[:, m, nd * SC:(nd + 1) * SC],
                            start=(m == 0), stop=(m == NMT - 1),
                        )
                for half in range(2):
                    # separate staging tiles per engine: a shared tile
                    # serialized the vector copy behind the scalar copy
                    o_a = outp.tile([P, SC], DT.float32, tag="outa",
                                    name=f"ota{st}_{half}")
                    o_b = outp.tile([P, SC], DT.float32, tag="outb",
                                    name=f"otb{st}_{half}")
                    nc.scalar.copy(o_a[:], o_ps[2 * half])
                    nc.vector.tensor_copy(o_b[:], o_ps[2 * half + 1])
                    eng = nc.sync if half == 0 else nc.gpsimd
                    c0 = half * 2 * SC
                    eng.dma_start(
                        out[st * P:(st + 1) * P, c0:c0 + SC], o_a[:])
                    eng.dma_start(
                        out[st * P:(st + 1) * P, c0 + SC:c0 + 2 * SC], o_b[:])

    nc.finalize()
    return nc


_NC_CACHE = None


def _get_nc():
    global _NC_CACHE
    if _NC_CACHE is None:
        _NC_CACHE = _build()
    return _NC_CACHE


def _in_maps(hidden_states, attention_mask, Wq, Wk, Wv, Wo, rel_emb):
    import ml_dtypes
    bf16 = ml_dtypes.bfloat16
    maps = []
    for c in range(NCORES):
        b, g = c // 4, c % 4
        hlo, hhi = g * HL, (g + 1) * HL
        erel = _bias_table(
            np.ascontiguousarray(rel_emb[:, hlo:hhi], dtype=np.float32))
        def shuf(w):  # [NDT*P, C] -> [P, NDT*C] partition-contiguous
            cc = w.shape[1]
            return np.ascontiguousarray(
                w.reshape(-1, P, cc).transpose(1, 0, 2).reshape(P, -1))
        # xt[p, qc, kd, j] = x[qc*512+j, kd*128+p]
        xtt = np.ascontiguousarray(
            hidden_states[b].reshape(NQC, SC, NDT, P).transpose(3, 0, 2, 1)
        ).astype(bf16)
        maps.append({
            "xt": xtt,
            "wq": shuf(Wq[:, hlo * D_KV:hhi * D_KV]).astype(bf16),
            "wk": shuf(Wk[:, hlo * D_KV:hhi * D_KV]).astype(bf16),
            "wv": shuf(Wv[:, hlo * D_KV:hhi * D_KV]).astype(bf16),
            "wo": shuf(Wo[hlo * D_KV:hhi * D_KV, :]).astype(bf16),
            "mask": np.ascontiguousarray(attention_mask[b, 0, 0, :]).astype(np.float32),
            "erel": erel,
        })
    return maps


def kernel(hidden_states, attention_mask, Wq, Wk, Wv, Wo, rel_emb, _trace=False,
           _trace_kwargs=None):
    hidden_states = np.asarray(hidden_states, dtype=np.float32)
    attention_mask = np.asarray(attention_mask, dtype=np.float32)
    Wq = np.asarray(Wq, dtype=np.float32)
    Wk = np.asarray(Wk, dtype=np.float32)
    Wv = np.asarray(Wv, dtype=np.float32)
    Wo = np.asarray(Wo, dtype=np.float32)
    rel_emb = np.asarray(rel_emb, dtype=np.float32)

    nc = _get_nc()
    maps = _in_maps(hidden_states, attention_mask, Wq, Wk, Wv, Wo, rel_emb)
    kw = dict(_trace_kwargs or {})
    res = run_bass_kernel_spmd(nc, maps, core_ids=list(range(NCORES)),
                               trace=_trace, **kw)
    kernel.last_results = res
    outp = np.empty((B, S, D), dtype=np.float32)
    for b in range(B):
        acc = np.asarray(res.results[4 * b]["out"], dtype=np.float32).copy()
        for g in range(1, 4):
            acc += np.asarray(res.results[4 * b + g]["out"], dtype=np.float32)
        outp[b] = acc
    return outp
